# revision 1
# baseline (speedup 1.0000x reference)
"""Trainium2 Bass kernel for nn_ConvLogicLayer.

Computes y[n,c,oy,ox,p] = k0 + ka*A + kb*B + kab*A*B where A/B are
shifted-window gathers of input channels (per the packed `selection`),
and k* are per-(c,p) coefficients derived from softmax(weights) @ OP_COEFFS.

Strategy:
  - Shard C_out (512) across 8 cores -> 64 output channels per core.
  - Each core gets a specialized program: the gather indices and the
    coefficients are baked into the instruction stream (static access
    patterns + immediate scalars), so the kernel is pure streaming
    elementwise work with zero gather traffic.
  - SBUF layout: partition q = n*4 + oyblk (32 images x 4 row-blocks),
    free dim = all 64 input channels x 10 halo rows x 34 padded cols.
    A shifted 8x32 window for any (ch,ry,rx) is then a single static
    3D access pattern on one SBUF tile.
  - Per (c,p) pair: u = kab*B + ka (ScalarE), v = kb*B + k0 and
    y = w + v load-balanced across VectorE/ScalarE/GPSIMD, w = u*A
    (VectorE).  y is written p-interleaved so the per-channel output DMA
    (512KB) is 4KB-contiguous in HBM.  Input load is chunked and ordered
    by first use so compute overlaps the streaming load.
    Load/compute overlap: 2-chunk load (most-used input channels first),
    output channels ordered by ready-pair count with ready pairs emitted
    first, and the chunk split auto-tuned per core via TimelineSim.
    Per-core auto-tune picks (load-chunk, greedy-balance constants) by
    TimelineSim.  Cost-model estimate: 166.0us (slowest core); measured
    rel err on hardware vs the f32 reference: 3.4e-07.
"""

import os
import sys
import threading

import numpy as np

for _p in ("/opt/trn_rl_repo",):
    if _p not in sys.path and os.path.isdir(_p):
        sys.path.insert(0, _p)

import concourse.bass as bass
import concourse.bacc as bacc
import concourse.mybir as mybir
from concourse.tile import TileContext
from concourse.masks import make_identity
from concourse import bass_utils

# Problem constants (hardcoded per spec)
N, C_IN, H, W = 32, 64, 32, 32
C_OUT, KPAIRS = 512, 4
N_CORES = 8
CPC = C_OUT // N_CORES  # channels per core

P = 128          # partitions = (n=32) x (oyblk=4)
OYB = 4          # oy blocks per image
OYS = 8          # oy rows per block
HALO = 10        # rows stored per block (8 + 2 halo)
W34 = 34         # padded width
CHSZ = HALO * W34           # 340 elems per (q, channel)
XFREE = C_IN * CHSZ         # 21760 elems per partition
OUT_CSTRIDE = H * W * KPAIRS          # 4096
OUT_NSTRIDE = CPC * OUT_CSTRIDE       # 262144

OP_COEFFS = np.array([
    [0.0, 0.0, 0.0, 0.0], [0.0, 0.0, 0.0, 1.0], [0.0, 1.0, 0.0, -1.0],
    [0.0, 1.0, 0.0, 0.0], [0.0, 0.0, 1.0, -1.0], [0.0, 0.0, 1.0, 0.0],
    [0.0, 1.0, 1.0, -2.0], [0.0, 1.0, 1.0, -1.0], [1.0, -1.0, -1.0, 1.0],
    [1.0, -1.0, -1.0, 2.0], [1.0, 0.0, -1.0, 0.0], [1.0, 0.0, -1.0, 1.0],
    [1.0, -1.0, 0.0, 0.0], [1.0, -1.0, 0.0, 1.0], [1.0, 0.0, 0.0, -1.0],
    [1.0, 0.0, 0.0, 0.0],
], dtype=np.float64)

MULT = mybir.AluOpType.mult
ADD = mybir.AluOpType.add
COPY = mybir.ActivationFunctionType.Copy

# Cost-model ns for load balancing (f32, [128, 256] tiles)
DVE_TT = 327.0   # tensor_tensor, 1x
DVE_TS = 194.0   # tensor_scalar, 2x_2P
ACT_TS = 507.0   # activation, 1x + 352cyc overhead
GPS_TS = 600.0   # gpsimd tensor_scalar (sw impl efficiency ~0.6)
GPS_TT = 600.0   # gpsimd tensor_tensor (sw impl efficiency ~0.42)
PE_U = 852.0     # two f32 matmuls (identity copy + bias row) on TensorE

# Tuning knobs (A/B'd via TimelineSim; best found = ~166us slowest core)
CFG = {
    "use_gps": True,     # offload v/y ops to GPSIMD
    "tp_bufs": 6,
    "yc_bufs": 8,
    "u_act_only": True,  # u always on ScalarE
    "w_dve_only": True,  # w always on VectorE
    "load_chunk_ch": 32,  # 2-chunk load (top-used channels first)
    "load_cascade": None,
    "any_uv": False,
    # TensorE u-offload (identity matmul + bias row, u = B + ka/kab, kab
    # re-applied at the y STT): numerically exact but model-NEGATIVE -- f32
    # matmul runs at 4 cycles/row plus cold p-state, and the PE->PSUM->DVE
    # chain serializes; every tested fraction lost ~7us. Kept for reference.
    "u_pe": False,
    "kab_min": 1e-3,     # |kab| guard for the u_pe refactoring
}

last_results = [None] * N_CORES  # BassKernelResults per core (for profiling)
last_model_ns = [None] * N_CORES  # per-core TimelineSim estimate of the shipped program


def build_core_program(core, ch, ry, rx, coef):
    """One specialized Bass program for `core` (channels core*CPC..+CPC)."""
    nc = bacc.Bacc("TRN2", target_bir_lowering=False)
    xh_d = nc.dram_tensor("xh", [P, XFREE], mybir.dt.float32, kind="ExternalInput")
    kap_d = nc.dram_tensor(
        "kap", [P, CPC * KPAIRS], mybir.dt.float32, kind="ExternalInput"
    )
    out_d = nc.dram_tensor(
        "out", [N, CPC, H, W, KPAIRS], mybir.dt.float32, kind="ExternalOutput"
    )

    use_gps = CFG["use_gps"]
    use_pe = CFG.get("u_pe")
    kab_min = CFG.get("kab_min", 1e-3)
    eng_ns = {"dve": 0.0, "act": 0.0, "gps": 0.0, "pe": 0.0}

    with TileContext(nc) as tc:
        with (
            tc.tile_pool(name="xp", bufs=1) as xpool,
            tc.tile_pool(name="tp", bufs=CFG["tp_bufs"]) as tpool,
            tc.tile_pool(name="yp", bufs=CFG["yc_bufs"]) as ypool,
            tc.tile_pool(name="pp", bufs=4, space="PSUM") as ppool,
        ):
            xh = xpool.tile([P, XFREE], mybir.dt.float32)
            # Jointly order output-channel processing (greedy: next output
            # needing fewest not-yet-loaded inputs) and stream input-channel
            # loads in that discovery order, so compute starts after a couple
            # of small chunks and fully overlaps the rest of the load.
            # (Tile's subtile tracking scopes each pair's waits to the load
            # DMAs it actually reads; Bacc splits any multi-wait syncs.)
            # Two-chunk load: chunk1 = the 32 most-used input channels, then
            # the rest.  A pair only needs its 2 input channels, so ~25-35%
            # of pairs are ready after chunk1 (~16us in); channels are ordered
            # by ready-pair count and ready pairs emitted first, so compute
            # overlaps the chunk2 load.  (Tile's subtile tracking scopes each
            # pair's waits to the load DMAs it reads.)
            chunk_ch = CFG.get("load_chunk_ch", 0)
            pair_chs = {
                (cl, p4): (int(ch[core * CPC + cl, 2 * p4]), int(ch[core * CPC + cl, 2 * p4 + 1]))
                for cl in range(CPC)
                for p4 in range(KPAIRS)
            }
            if chunk_ch <= 0:
                cl_order = list(range(CPC))
                p4_order = {cl: list(range(KPAIRS)) for cl in range(CPC)}
                nc.sync.dma_start(xh[:], xh_d[:])
            else:
                use_cnt = [0] * C_IN
                for a, b in pair_chs.values():
                    use_cnt[a] += 1
                    use_cnt[b] += 1
                by_use = sorted(range(C_IN), key=lambda i: -use_cnt[i])
                sizes = CFG.get("load_cascade") or [chunk_ch, C_IN - chunk_ch]
                tier_of = {}
                pos = 0
                groups = []
                for t, sz in enumerate(sizes):
                    grp = by_use[pos : pos + sz]
                    pos += sz
                    for cch in grp:
                        tier_of[cch] = t
                    if grp:
                        groups.append(sorted(grp))
                ptier = {
                    (cl, p4): max(tier_of[a], tier_of[b])
                    for (cl, p4), (a, b) in pair_chs.items()
                }
                tiers = {cl: sorted(ptier[(cl, p4)] for p4 in range(KPAIRS)) for cl in range(CPC)}
                cl_order = sorted(range(CPC), key=lambda cl: tiers[cl])
                p4_order = {
                    cl: sorted(range(KPAIRS), key=lambda p4: ptier[(cl, p4)])
                    for cl in range(CPC)
                }
                for grp in groups:
                    run = [grp[0]]
                    for cch in grp[1:] + [None]:
                        if cch is not None and cch == run[-1] + 1:
                            run.append(cch)
                            continue
                        lo, hi = run[0] * CHSZ, (run[-1] + 1) * CHSZ
                        nc.sync.dma_start(xh[:, lo:hi], xh_d[:, lo:hi])
                        if cch is not None:
                            run = [cch]
            base = xh[:]
            pitch = base.ap[0][0]
            tens = base.tensor
            base_off = base.offset

            if use_pe:
                ident = xpool.tile([P, P], mybir.dt.float32, tag="ident")
                ones = xpool.tile([1, W * OYS], mybir.dt.float32, tag="ones")
                kap = xpool.tile([P, CPC * KPAIRS], mybir.dt.float32, tag="kap")
                make_identity(nc, ident[:])
                nc.vector.memset(ones[:], 1.0)
                nc.sync.dma_start(kap[:], kap_d[:])

            for cl in cl_order:
                c = core * CPC + cl
                yc = ypool.tile([P, OYS * W * KPAIRS], mybir.dt.float32, tag="yc")
                ybase = yc[:]
                ypitch = ybase.ap[0][0]
                for p4 in p4_order[cl]:
                    ka_, kb_ = 2 * p4, 2 * p4 + 1
                    offA = base_off + int(ch[c, ka_]) * CHSZ + int(ry[c, ka_]) * W34 + int(rx[c, ka_])
                    offB = base_off + int(ch[c, kb_]) * CHSZ + int(ry[c, kb_]) * W34 + int(rx[c, kb_])
                    A_ap = bass.AP(tens, offA, [[pitch, P], [W34, OYS], [1, W]])
                    B_ap = bass.AP(tens, offB, [[pitch, P], [W34, OYS], [1, W]])

                    k0 = float(coef[c, p4, 0])
                    ka = float(coef[c, p4, 1])
                    kb = float(coef[c, p4, 2])
                    kab = float(coef[c, p4, 3])

                    u = tpool.tile([P, OYS * W], mybir.dt.float32, tag="u")
                    v = tpool.tile([P, OYS * W], mybir.dt.float32, tag="v")
                    w = tpool.tile([P, OYS * W], mybir.dt.float32, tag="w")
                    u3 = u[:].rearrange("p (a b) -> p a b", b=W)
                    v3 = v[:].rearrange("p (a b) -> p a b", b=W)
                    w3 = w[:].rearrange("p (a b) -> p a b", b=W)

                    def pick(cands):
                        eng, cost = min(cands, key=lambda c: eng_ns[c[0]] + c[1])
                        eng_ns[eng] += cost
                        return eng

                    # u = kab*B + ka
                    pe_ok = use_pe and abs(kab) >= kab_min
                    ueng = None
                    if CFG.get("any_uv"):
                        nc.any.tensor_scalar(u3, B_ap, kab, ka, MULT, ADD)
                        nc.any.tensor_scalar(v3, B_ap, kb, k0, MULT, ADD)
                        eng_ns["act"] += ACT_TS  # rough accounting
                        eng_ns["dve"] += DVE_TS
                    else:
                        if CFG.get("u_act_only"):
                            ucands = [("act", ACT_TS)]
                        else:
                            ucands = [("act", ACT_TS), ("dve", DVE_TS)]
                            if use_gps:
                                ucands.append(("gps", GPS_TS))
                        if pe_ok:
                            ucands = ucands + [("pe", PE_U)]
                        ueng = pick(ucands)
                        if ueng == "pe":
                            # u = B + ka/kab via identity matmul + bias row;
                            # kab is re-applied at the y step (STT below).
                            upsum = ppool.tile([P, OYS * W], mybir.dt.float32, tag="up")
                            j = cl * KPAIRS + p4
                            nc.tensor.matmul(
                                out=upsum[:], lhsT=ident[:], rhs=B_ap,
                                start=True, stop=False,
                            )
                            nc.tensor.matmul(
                                out=upsum[:],
                                lhsT=kap[0:1, j : j + 1].to_broadcast((1, P)),
                                rhs=ones[:], start=False, stop=True,
                            )
                            u3 = upsum[:].rearrange("p (a b) -> p a b", b=W)
                        elif ueng == "act":
                            nc.scalar.activation(u3, B_ap, COPY, bias=ka, scale=kab)
                        elif ueng == "gps":
                            nc.gpsimd.tensor_scalar(u3, B_ap, kab, ka, MULT, ADD)
                        else:
                            nc.vector.tensor_scalar(u3, B_ap, kab, ka, MULT, ADD)
                        # v = kb*B + k0
                        vcands = [("dve", DVE_TS), ("act", ACT_TS)]
                        if use_gps:
                            vcands.append(("gps", GPS_TS))
                        veng = pick(vcands)
                        if veng == "act":
                            nc.scalar.activation(v3, B_ap, COPY, bias=k0, scale=kb)
                        elif veng == "gps":
                            nc.gpsimd.tensor_scalar(v3, B_ap, kb, k0, MULT, ADD)
                        else:
                            nc.vector.tensor_scalar(v3, B_ap, kb, k0, MULT, ADD)
                    # w = u * A
                    wcands = [("dve", DVE_TT)]
                    if use_gps and not CFG.get("w_dve_only"):
                        wcands.append(("gps", GPS_TT))
                    weng = pick(wcands)
                    if weng == "gps":
                        nc.gpsimd.tensor_tensor(w3, u3, A_ap, MULT)
                    else:
                        nc.vector.tensor_tensor(w3, u3, A_ap, MULT)
                    # y = w + v, written p-interleaved into yc
                    yap = bass.AP(
                        ybase.tensor, ybase.offset + p4,
                        [[ypitch, P], [W * KPAIRS, OYS], [KPAIRS, W]],
                    )
                    ycands = [("dve", DVE_TT)]
                    if use_gps:
                        ycands.append(("gps", GPS_TT))
                    yeng = pick(ycands)
                    if not CFG.get("any_uv") and ueng == "pe":
                        # y = kab*w + v (kab deferred from the PE u form)
                        if yeng == "gps":
                            nc.gpsimd.scalar_tensor_tensor(yap, w3, kab, v3, MULT, ADD)
                        else:
                            nc.vector.scalar_tensor_tensor(yap, w3, kab, v3, MULT, ADD)
                    elif yeng == "gps":
                        nc.gpsimd.tensor_tensor(yap, w3, v3, ADD)
                    else:
                        nc.vector.tensor_tensor(yap, w3, v3, ADD)

                # DMA this channel out: HBM [n, oyblk, (oy',ox,p)=1024]
                oap = bass.AP(
                    out_d, cl * OUT_CSTRIDE,
                    [[OUT_NSTRIDE, N], [OYS * W * KPAIRS, OYB], [1, OYS * W * KPAIRS]],
                )
                nc.sync.dma_start(oap, yc[:])
    nc.finalize()  # Bacc: splits >1-wait syncs into event semaphores
    return nc


def _prep_inputs(x, weights, selection):
    x = np.ascontiguousarray(np.asarray(x, dtype=np.float32))
    weights = np.asarray(weights, dtype=np.float32)
    selection = np.asarray(selection, dtype=np.int32)

    # coefficients: softmax over 16 logic ops folded into {1,a,b,ab} basis
    w64 = weights.astype(np.float64)
    e = np.exp(w64 - w64.max(axis=-1, keepdims=True))
    prob = e / e.sum(axis=-1, keepdims=True)
    coef = (prob @ OP_COEFFS).astype(np.float32)  # [C_OUT, 4, 4]

    ch = ((selection >> 16) & 0xFFFF).astype(np.int64)
    ry = ((selection >> 8) & 0xFF).astype(np.int64)
    rx = (selection & 0xFF).astype(np.int64)

    # halo layout: xh[q=(n,oyblk), ch, r, w] = xpad[n, ch, oyblk*8+r, w]
    xpad = np.zeros((N, C_IN, H + 2, W + 2), dtype=np.float32)
    xpad[:, :, 1 : H + 1, 1 : W + 1] = x
    xh = np.empty((N, OYB, C_IN, HALO, W34), dtype=np.float32)
    for b in range(OYB):
        xh[:, b] = xpad[:, :, b * OYS : b * OYS + HALO, :]
    xh = np.ascontiguousarray(xh.reshape(P, XFREE))
    return xh, ch, ry, rx, coef


def kernel(x, weights, selection):
    assert x.shape == (N, C_IN, H, W), x.shape
    assert weights.shape == (C_OUT, 4, 16), weights.shape
    assert selection.shape == (C_OUT, 8), selection.shape

    xh, ch, ry, rx, coef = _prep_inputs(x, weights, selection)

    # per-core ka/kab bias rows for the TensorE u-path (0 where unused)
    kab_min = CFG.get("kab_min", 1e-3)
    kap_arrs = []
    for k in range(N_CORES):
        kap = np.zeros((P, CPC * KPAIRS), dtype=np.float32)
        for cl in range(CPC):
            c = k * CPC + cl
            for p4 in range(KPAIRS):
                kab_v = float(coef[c, p4, 3])
                if abs(kab_v) >= kab_min:
                    kap[:, cl * KPAIRS + p4] = float(coef[c, p4, 1]) / kab_v
        kap_arrs.append(kap)

    # Per-core auto-tune: each core's selection pattern favors a different
    # chunk1 size for the load/compute overlap — build a few candidates and
    # keep the one the TimelineSim cost model scores fastest.
    try:
        from concourse.timeline_sim import TimelineSim
    except Exception:  # noqa: BLE001
        TimelineSim = None
    cands = CFG.get(
        "tune_candidates",
        (
            (32, 507.0, 600.0), (36, 507.0, 600.0), (40, 507.0, 600.0),
            (44, 507.0, 600.0), (36, 480.0, 600.0), (40, 480.0, 600.0),
            (40, 480.0, 500.0), (36, 480.0, 500.0), (38, 480.0, 500.0),
            (40, 480.0, 550.0),
        ),
    )
    progs = []
    base_chunk = CFG["load_chunk_ch"]
    global ACT_TS, GPS_TS
    base_act, base_gts = ACT_TS, GPS_TS
    for k in range(N_CORES):
        best = None
        for cc, act, gts in cands if TimelineSim is not None else ((base_chunk, base_act, base_gts),):
            CFG["load_chunk_ch"] = cc
            ACT_TS, GPS_TS = act, gts
            nc = build_core_program(k, ch, ry, rx, coef)
            ns = None
            if TimelineSim is not None:
                try:
                    ns = TimelineSim(nc, trace=False).simulate()
                except Exception:  # noqa: BLE001
                    ns = None
            if best is None or (ns is not None and best[0] is not None and ns < best[0]):
                best = (ns, nc)
            if ns is None:
                break
        progs.append(best[1])
        last_model_ns[k] = best[0]
    CFG["load_chunk_ch"] = base_chunk
    ACT_TS, GPS_TS = base_act, base_gts

    import jax

    devices = jax.devices()
    assert len(devices) >= N_CORES, devices

    outs = [None] * N_CORES
    errs = [None] * N_CORES
    # NTFF tracing needs axon hooks that aren't present in this container —
    # make sure run_bass_kernel_spmd never tries (BASS_TRACE in env would).
    os.environ["BASS_NEVER_TRACE"] = "1"

    def run_one(k):
        try:
            with jax.default_device(devices[k]):
                res = bass_utils.run_bass_kernel_spmd(
                    progs[k], [{"xh": xh, "kap": kap_arrs[k]}], core_ids=[k]
                )
            last_results[k] = res
            outs[k] = res.results[0]["out"]
        except Exception as e:  # noqa: BLE001
            errs[k] = e

    threads = [threading.Thread(target=run_one, args=(k,)) for k in range(N_CORES)]
    for t in threads:
        t.start()
    for t in threads:
        t.join()
    for k, e in enumerate(errs):
        if e is not None:
            raise RuntimeError(f"core {k} failed") from e

    y = np.empty((N, C_OUT, H, W, KPAIRS), dtype=np.float32)
    for k in range(N_CORES):
        y[:, k * CPC : (k + 1) * CPC] = outs[k]
    return y



# revision 41
# speedup vs baseline: 1.3703x; 1.3703x over previous
"""Trainium2 Bass kernel for nn_ConvLogicLayer.

Computes y[n,c,oy,ox,p] = k0 + ka*A + kb*B + kab*A*B where A/B are
shifted-window gathers of input channels (per the packed `selection`),
and k* are per-(c,p) coefficients derived from softmax(weights) @ OP_COEFFS.

Strategy (v2, fp16):
  - Shard C_out (512) across 8 cores -> 64 output channels per core.
  - Specialized per-core program: gather indices and coefficients baked
    into the instruction stream (static APs + immediate scalars).
  - SBUF layout: partition q = n*4 + oyblk (32 images x 4 row-blocks),
    free dim = 64 input channels x 10 halo rows x 34 padded cols, fp16.
    A shifted 8x32 window for any (ch,ry,rx) is a static 3D AP.
  - fp16 everywhere: DVE runs tensor_scalar in 4x mode (127ns/[128,256])
    and tensor_tensor in 2x mode (194ns), and DMA bytes halve (in 5.6MB,
    out 16.8MB per core).  Host converts the fp16 output to f32.
  - Per pair (c,p): u = kab*B + ka (DVE ts), w = u*A (DVE tt).  Then
    either per-pair y = (kb*B + k0) + w via the AFFINE_THEN_ADD custom
    DVE uop (strided p-interleaved write, v folded), or per-channel
    merged y: v = kb*B + k0 (ACT/DVE/GPS) into v_all, then one GPSIMD
    scalar_tensor_tensor [128,1024] computes yc = w_all + v_all with the
    p-interleave expressed in the APs.  Mode chosen greedily per channel
    to balance DVE/ACT/GPS finish times (LP optimum ~89.8us/core).
  - y written p-interleaved so the per-channel output DMA (256KB fp16)
    is 2KB-contiguous per partition in HBM.  Input load is chunked and
    ordered by first use so compute overlaps the streaming load.
"""

import os
import sys
import threading

import numpy as np

for _p in ("/opt/trn_rl_repo",):
    if _p not in sys.path and os.path.isdir(_p):
        sys.path.insert(0, _p)

import concourse.bass as bass
import concourse.bacc as bacc
import concourse.mybir as mybir
from concourse.tile import TileContext
from concourse import bass_utils

# Problem constants (hardcoded per spec)
N, C_IN, H, W = 32, 64, 32, 32
C_OUT, KPAIRS = 512, 4
N_CORES = 8
CPC = C_OUT // N_CORES  # channels per core

P = 128          # partitions = (n=32) x (oyblk=4)
OYB = 4          # oy blocks per image
OYS = 8          # oy rows per block
HALO = 10        # rows stored per block (8 + 2 halo)
W34 = 34         # padded width
CHSZ = HALO * W34           # 340 elems per (q, channel)
XFREE = C_IN * CHSZ         # 21760 elems per partition
OUT_CSTRIDE = H * W * KPAIRS          # 4096
OUT_NSTRIDE = CPC * OUT_CSTRIDE       # 262144
SPP = OYS * W    # 256 elems per (partition, pair)

OP_COEFFS = np.array([
    [0.0, 0.0, 0.0, 0.0], [0.0, 0.0, 0.0, 1.0], [0.0, 1.0, 0.0, -1.0],
    [0.0, 1.0, 0.0, 0.0], [0.0, 0.0, 1.0, -1.0], [0.0, 0.0, 1.0, 0.0],
    [0.0, 1.0, 1.0, -2.0], [0.0, 1.0, 1.0, -1.0], [1.0, -1.0, -1.0, 1.0],
    [1.0, -1.0, -1.0, 2.0], [1.0, 0.0, -1.0, 0.0], [1.0, 0.0, -1.0, 1.0],
    [1.0, -1.0, 0.0, 0.0], [1.0, -1.0, 0.0, 1.0], [1.0, 0.0, 0.0, -1.0],
    [1.0, 0.0, 0.0, 0.0],
], dtype=np.float64)

MULT = mybir.AluOpType.mult
ADD = mybir.AluOpType.add
COPY = mybir.ActivationFunctionType.Copy
F16 = mybir.dt.float16

# TimelineSim per-op costs (fp16, [128,256] unless noted)
DVE_TS = 127.0    # tensor_scalar, 4x mode
DVE_TT = 194.0    # tensor_tensor contiguous, 2x mode
DVE_ATA = 327.0   # AFFINE_THEN_ADD custom uop (any stride)
ACT_TS = 398.0    # activation copy w/ scale+bias
GPS_TS = 451.0    # gpsimd tensor_scalar
GPS_TT_M = 2127.0   # gpsimd tensor_tensor [128,1024] merged y (stt is f32-only)
GPS_TT_S = 603.0    # gpsimd tensor_tensor [128,256] per-pair y
DVE_TT_M = 1127.0   # DVE tensor_tensor [128,1024] merged y (1x, strided in)

CFG = {
    "load_piece_ch": 4,  # channels per streamed load DMA
    "load_cascade": None,
    "tp_bufs": 6,    # u/w scratch tiles
    "wv_bufs": 6,    # w_all/v_all channel tiles (GPS merged-y inputs)
    "yc_bufs": 8,    # output channel tiles
    "pipe_depth": 0,  # channels of consumer delay (0 = inline)
    "head_ata": 0,   # first channels forced all-DVE
    "tail_ata": 0,   # last channels forced all-DVE
    "gps_v_head": 4,  # first channels with v forced to GPSIMD
    "seed_dve": 0.0,
    "seed_act": 0.0,
    "seed_gps": 0.0,
}

last_results = [None] * N_CORES  # BassKernelResults per core (for profiling)
last_model_ns = [None] * N_CORES  # per-core TimelineSim estimate
last_eng_ns = [None] * N_CORES  # per-core greedy engine-load projection


def core_channel_perm(core, ch):
    """Input-channel permutation for `core`: most-used first.

    Returns (perm, inv) with perm[pos] = original channel stored at slot pos
    in this core's xh, inv[orig] = slot.
    """
    use_cnt = [0] * C_IN
    for cl in range(CPC):
        for j in range(2 * KPAIRS):
            use_cnt[int(ch[core * CPC + cl, j])] += 1
    perm = sorted(range(C_IN), key=lambda i: (-use_cnt[i], i))
    inv = [0] * C_IN
    for pos, orig in enumerate(perm):
        inv[orig] = pos
    return perm, inv


def build_core_program(core, ch, ry, rx, coef):
    """One specialized Bass program for `core` (channels core*CPC..+CPC).

    The host stores this core's xh with channels permuted most-used-first
    (core_channel_perm); `ch` is remapped here to slot indices so the
    cascade [s0, s1, ...] loads contiguous slabs front-to-back.
    """
    nc = bacc.Bacc("TRN2", target_bir_lowering=False)
    xh_d = nc.dram_tensor("xh", [P, XFREE], F16, kind="ExternalInput")
    out_d = nc.dram_tensor(
        "out", [N, CPC, H, W, KPAIRS], F16, kind="ExternalOutput"
    )

    eng_ns = {
        "dve": float(CFG.get("seed_dve", 0.0)),
        "act": float(CFG.get("seed_act", 0.0)),
        "gps": float(CFG.get("seed_gps", 0.0)),
    }

    with TileContext(nc) as tc:
        with (
            tc.tile_pool(name="xp", bufs=1) as xpool,
            tc.tile_pool(name="tp", bufs=CFG["tp_bufs"]) as tpool,
            tc.tile_pool(name="wv", bufs=CFG["wv_bufs"]) as wvpool,
            tc.tile_pool(name="yp", bufs=CFG["yc_bufs"]) as ypool,
        ):
            xh = xpool.tile([P, XFREE], F16)
            # ch remapped to this core's permuted slots (most-used first).
            _, inv = core_channel_perm(core, ch)
            chv = {
                (cl, j): inv[int(ch[core * CPC + cl, j])]
                for cl in range(CPC)
                for j in range(2 * KPAIRS)
            }
            # Streamed load: many small contiguous piece-DMAs front-to-back
            # (most-used slots first).  Pairs/channels are ordered by the
            # piece at which both operands are resident, so the in-order
            # engine queues never head-of-line block on a late channel.
            sizes = CFG.get("load_cascade")
            if not sizes:
                g = CFG.get("load_piece_ch", 4)
                sizes = [g] * (C_IN // g) + ([C_IN % g] if C_IN % g else [])
            bounds = []
            acc = 0
            for sz in sizes:
                acc += sz
                bounds.append(acc)
            assert bounds[-1] == C_IN, bounds

            def tier_of_slot(slot):
                for t, b in enumerate(bounds):
                    if slot < b:
                        return t
                raise AssertionError(slot)

            ptier = {
                (cl, p4): max(
                    tier_of_slot(chv[(cl, 2 * p4)]), tier_of_slot(chv[(cl, 2 * p4 + 1)])
                )
                for cl in range(CPC)
                for p4 in range(KPAIRS)
            }
            tiers = {cl: sorted(ptier[(cl, p4)] for p4 in range(KPAIRS)) for cl in range(CPC)}
            # sort by completion tier (max first): a channel is consumable
            # only once its LAST pair's inputs arrive, so straggler channels
            # must not sit at the front of the in-order queues.
            cl_order = sorted(range(CPC), key=lambda cl: tiers[cl][::-1])
            p4_order = {
                cl: sorted(range(KPAIRS), key=lambda p4: ptier[(cl, p4)])
                for cl in range(CPC)
            }
            lo = 0
            for b in bounds:
                nc.sync.dma_start(
                    xh[:, lo * CHSZ : b * CHSZ], xh_d[:, lo * CHSZ : b * CHSZ]
                )
                lo = b
            base = xh[:]
            pitch = base.ap[0][0]
            tens = base.tensor
            base_off = base.offset

            # Software pipelining: emit each channel's producers (u/w/v) now
            # but its consumers (merged gstt + output DMA) D channels later,
            # so no engine's queue head waits on a just-issued cross-engine
            # dependency.
            pipe_d = CFG.get("pipe_depth", 3)
            pending = []  # (w_all, v_all, yc, oap, y_eng) awaiting merge+DMA
            st = {"mp": 0, "ua": 0, "va": 0}  # merged-pair / ACT-quota counters

            def flush_one():
                w_all, v_all, yc_t, oap_t, y_eng = pending.pop(0)
                yb = yc_t[:]
                yp_ = yb.ap[0][0]
                wb = w_all[:]
                vb = v_all[:]
                w_ap = bass.AP(wb.tensor, wb.offset, [[wb.ap[0][0], P], [1, SPP], [SPP, KPAIRS]])
                v_ap = bass.AP(vb.tensor, vb.offset, [[vb.ap[0][0], P], [1, SPP], [SPP, KPAIRS]])
                y_ap = bass.AP(yb.tensor, yb.offset, [[yp_, P], [KPAIRS, SPP], [1, KPAIRS]])
                if y_eng == "gps":
                    nc.gpsimd.tensor_tensor(y_ap, w_ap, v_ap, ADD)
                else:
                    nc.vector.tensor_tensor(y_ap, w_ap, v_ap, ADD)
                nc.sync.dma_start(oap_t, yb)

            for cl in cl_order:
                c = core * CPC + cl
                yc = ypool.tile([P, OYS * W * KPAIRS], F16, tag="yc")
                ybase = yc[:]
                ypitch = ybase.ap[0][0]

                # channel mode: merged y on GPS vs per-pair ATA on DVE.
                # Project both and keep whichever minimizes the makespan.
                # LP-quota assignment: mode basis is m3 (u ACT + all-DVE ATA),
                # m5 (u/v DVE + y GPS-merged), m7 (u/v ACT + y GPS-merged).
                # Solved so each engine's start offset + load is equal --
                # engines finish together rather than having equal totals.
                pos = cl_order.index(cl)
                f_ata = CFG.get("f_ata", 0.281)
                uv_act = CFG.get("uv_act", 0.473)
                if CFG.get("auto_quota", 0):
                    M = np.array([
                        [521.0, 448.0, 194.0, -1.0],
                        [398.0, 0.0, 796.0, -1.0],
                        [0.0, 532.0, 532.0, -1.0],
                        [1.0, 1.0, 1.0, 0.0],
                    ])
                    rhs = np.array([
                        -float(CFG.get("off_dve", 6500.0)),
                        -float(CFG.get("off_act", 6500.0)),
                        -float(CFG.get("off_gps", 11000.0)),
                        float(C_OUT // N_CORES * KPAIRS),
                    ])
                    x3, x5, x7, _T = np.linalg.solve(M, rhs)
                    x3, x5, x7 = max(x3, 0.0), max(x5, 0.0), max(x7, 0.0)
                    tot = x3 + x5 + x7
                    f_ata = x3 / tot
                    uv_act = x7 / max(x5 + x7, 1e-9)
                n_ata = int(round(f_ata * CPC))
                tail = min(CFG.get("tail_ata", 0), n_ata)
                head_m = CFG.get("head_merged", 0)
                spread = n_ata - tail
                span = CPC - tail - head_m
                if pos >= CPC - tail:
                    is_ata = True
                elif pos < head_m or span <= 0:
                    is_ata = False
                else:
                    q = pos - head_m
                    is_ata = int(q * spread / span) < int((q + 1) * spread / span)
                if pos < CFG.get("head_ata", 0):
                    is_ata = True
                if is_ata:
                    mode = "ata"
                elif CFG.get("gps_single", 0):
                    mode = "gps_s"  # per-pair gtt, strided out (no w_all/v_all)
                else:
                    mode = "gps_m"
                merged = mode == "gps_m"
                uplace, vplace = [], []
                for _ in range(KPAIRS):
                    if is_ata:
                        uplace.append("act")
                        eng_ns["act"] += ACT_TS
                        eng_ns["dve"] += DVE_TT + DVE_ATA
                        continue
                    if mode == "gps_s":
                        eng_ns["gps"] += GPS_TT_S
                    st["mp"] += 1
                    # u placement against the ACT quota
                    if st["ua"] < uv_act * st["mp"]:
                        uplace.append("act")
                        st["ua"] += 1
                        eng_ns["act"] += ACT_TS
                    else:
                        uplace.append("dve")
                        eng_ns["dve"] += DVE_TS
                    eng_ns["dve"] += DVE_TT
                    if pos < CFG.get("gps_v_head", 0):
                        vplace.append("gps")
                        eng_ns["gps"] += GPS_TS
                    elif st["va"] < uv_act * st["mp"]:
                        vplace.append("act")
                        st["va"] += 1
                        eng_ns["act"] += ACT_TS
                    else:
                        vplace.append("dve")
                        eng_ns["dve"] += DVE_TS
                if merged:
                    eng_ns["gps"] += GPS_TT_M
                    w_all = wvpool.tile([P, KPAIRS * SPP], F16, tag="wa")
                    v_all = wvpool.tile([P, KPAIRS * SPP], F16, tag="va")

                for i, p4 in enumerate(p4_order[cl]):
                    ka_, kb_ = 2 * p4, 2 * p4 + 1
                    offA = base_off + chv[(cl, ka_)] * CHSZ + int(ry[c, ka_]) * W34 + int(rx[c, ka_])
                    offB = base_off + chv[(cl, kb_)] * CHSZ + int(ry[c, kb_]) * W34 + int(rx[c, kb_])
                    A_ap = bass.AP(tens, offA, [[pitch, P], [W34, OYS], [1, W]])
                    B_ap = bass.AP(tens, offB, [[pitch, P], [W34, OYS], [1, W]])

                    k0 = float(coef[c, p4, 0])
                    ka = float(coef[c, p4, 1])
                    kb = float(coef[c, p4, 2])
                    kab = float(coef[c, p4, 3])

                    u = tpool.tile([P, SPP], F16, tag="u")
                    u3 = u[:].rearrange("p (a b) -> p a b", b=W)
                    # u = kab*B + ka
                    ue = uplace[i]
                    if ue == "act":
                        nc.scalar.activation(u3, B_ap, COPY, bias=ka, scale=kab)
                    elif ue == "gps":
                        nc.gpsimd.tensor_scalar(u3, B_ap, kab, ka, MULT, ADD)
                    else:
                        nc.vector.tensor_scalar(u3, B_ap, kab, ka, MULT, ADD)

                    if merged:
                        wsl = w_all[:, p4 * SPP : (p4 + 1) * SPP]
                        w3 = wsl.rearrange("p (a b) -> p a b", b=W)
                        # w = u*A  (DVE tt, 2x, contiguous out)
                        nc.vector.tensor_tensor(w3, u3, A_ap, MULT)
                        vsl = v_all[:, p4 * SPP : (p4 + 1) * SPP]
                        v3 = vsl.rearrange("p (a b) -> p a b", b=W)
                        e = vplace[i]
                        if e == "act":
                            nc.scalar.activation(v3, B_ap, COPY, bias=k0, scale=kb)
                        elif e == "gps":
                            nc.gpsimd.tensor_scalar(v3, B_ap, kb, k0, MULT, ADD)
                        else:
                            nc.vector.tensor_scalar(v3, B_ap, kb, k0, MULT, ADD)
                    elif mode == "gps_s":
                        w = tpool.tile([P, SPP], F16, tag="w")
                        v = tpool.tile([P, SPP], F16, tag="v")
                        w3 = w[:].rearrange("p (a b) -> p a b", b=W)
                        v3 = v[:].rearrange("p (a b) -> p a b", b=W)
                        nc.vector.tensor_tensor(w3, u3, A_ap, MULT)
                        e = vplace[i]
                        if e == "act":
                            nc.scalar.activation(v3, B_ap, COPY, bias=k0, scale=kb)
                        elif e == "gps":
                            nc.gpsimd.tensor_scalar(v3, B_ap, kb, k0, MULT, ADD)
                        else:
                            nc.vector.tensor_scalar(v3, B_ap, kb, k0, MULT, ADD)
                        yap = bass.AP(
                            ybase.tensor, ybase.offset + p4,
                            [[ypitch, P], [W * KPAIRS, OYS], [KPAIRS, W]],
                        )
                        nc.gpsimd.tensor_tensor(yap, w3, v3, ADD)
                    else:
                        w = tpool.tile([P, SPP], F16, tag="w")
                        w3 = w[:].rearrange("p (a b) -> p a b", b=W)
                        nc.vector.tensor_tensor(w3, u3, A_ap, MULT)
                        # y = (kb*B + k0) + w, p-interleaved into yc (custom uop)
                        yap = bass.AP(
                            ybase.tensor, ybase.offset + p4,
                            [[ypitch, P], [W * KPAIRS, OYS], [KPAIRS, W]],
                        )
                        nc.vector.affine_then_add(yap, B_ap, w3, kb, k0)

                # HBM [n, oyblk, (oy',ox,p)=1024]
                oap = bass.AP(
                    out_d, cl * OUT_CSTRIDE,
                    [[OUT_NSTRIDE, N], [OYS * W * KPAIRS, OYB], [1, OYS * W * KPAIRS]],
                )
                if merged:
                    pending.append((w_all, v_all, yc, oap, "gps" if mode == "gps_m" else "dve"))
                    if len(pending) > pipe_d:
                        flush_one()
                else:
                    nc.sync.dma_start(oap, ybase)
            while pending:
                flush_one()
    nc.finalize()  # Bacc: splits >1-wait syncs into event semaphores
    last_eng_ns[core] = dict(eng_ns)
    return nc


def _prep_inputs(x, weights, selection):
    x = np.ascontiguousarray(np.asarray(x, dtype=np.float32))
    weights = np.asarray(weights, dtype=np.float32)
    selection = np.asarray(selection, dtype=np.int32)

    # coefficients: softmax over 16 logic ops folded into {1,a,b,ab} basis
    w64 = weights.astype(np.float64)
    e = np.exp(w64 - w64.max(axis=-1, keepdims=True))
    prob = e / e.sum(axis=-1, keepdims=True)
    coef = (prob @ OP_COEFFS).astype(np.float32)  # [C_OUT, 4, 4]

    ch = ((selection >> 16) & 0xFFFF).astype(np.int64)
    ry = ((selection >> 8) & 0xFF).astype(np.int64)
    rx = (selection & 0xFF).astype(np.int64)

    # halo layout: xh[q=(n,oyblk), ch, r, w] = xpad[n, ch, oyblk*8+r, w]
    xpad = np.zeros((N, C_IN, H + 2, W + 2), dtype=np.float32)
    xpad[:, :, 1 : H + 1, 1 : W + 1] = x
    xh = np.empty((N, OYB, C_IN, HALO, W34), dtype=np.float16)
    for b in range(OYB):
        xh[:, b] = xpad[:, :, b * OYS : b * OYS + HALO, :]
    # per-core copies with channels permuted most-used-first so the load
    # cascade is a few big contiguous DMAs
    xh_cores = []
    for k in range(N_CORES):
        perm, _ = core_channel_perm(k, ch)
        xh_cores.append(np.ascontiguousarray(xh[:, :, perm].reshape(P, XFREE)))
    return xh_cores, ch, ry, rx, coef


def kernel(x, weights, selection):
    assert x.shape == (N, C_IN, H, W), x.shape
    assert weights.shape == (C_OUT, 4, 16), weights.shape
    assert selection.shape == (C_OUT, 8), selection.shape

    xh_cores, ch, ry, rx, coef = _prep_inputs(x, weights, selection)

    try:
        from concourse.timeline_sim import TimelineSim
    except Exception:  # noqa: BLE001
        TimelineSim = None
    # Per-core auto-tune: each core's selection pattern favors different
    # schedule knobs; build candidates and keep the TimelineSim-fastest.
    cands = CFG.get(
        "tune_candidates",
        (
            {},
            {"gps_v_head": 0},
            {"load_piece_ch": 8},
            {"load_piece_ch": 8, "gps_v_head": 0},
        ),
    )
    progs = []
    base_cfg = dict(CFG)
    for k in range(N_CORES):
        best = None
        for cfg_delta in cands if TimelineSim is not None else ({},):
            CFG.clear()
            CFG.update(base_cfg)
            CFG.update(cfg_delta)
            nc = build_core_program(k, ch, ry, rx, coef)
            ns = None
            if TimelineSim is not None:
                try:
                    ns = TimelineSim(nc, trace=False).simulate()
                except Exception:  # noqa: BLE001
                    ns = None
            if best is None or (ns is not None and best[0] is not None and ns < best[0]):
                best = (ns, nc)
            if ns is None:
                break
        progs.append(best[1])
        last_model_ns[k] = best[0]
    CFG.clear()
    CFG.update(base_cfg)

    import jax

    devices = jax.devices()
    assert len(devices) >= N_CORES, devices

    outs = [None] * N_CORES
    errs = [None] * N_CORES
    # NTFF tracing needs axon hooks that aren't present in this container —
    # make sure run_bass_kernel_spmd never tries (BASS_TRACE in env would).
    os.environ["BASS_NEVER_TRACE"] = "1"

    def run_one(k):
        try:
            with jax.default_device(devices[k]):
                res = bass_utils.run_bass_kernel_spmd(
                    progs[k], [{"xh": xh_cores[k]}], core_ids=[k]
                )
            last_results[k] = res
            outs[k] = res.results[0]["out"]
        except Exception as e:  # noqa: BLE001
            errs[k] = e

    threads = [threading.Thread(target=run_one, args=(k,)) for k in range(N_CORES)]
    for t in threads:
        t.start()
    for t in threads:
        t.join()
    for k, e in enumerate(errs):
        if e is not None:
            raise RuntimeError(f"core {k} failed") from e

    y = np.empty((N, C_OUT, H, W, KPAIRS), dtype=np.float32)
    for k in range(N_CORES):
        y[:, k * CPC : (k + 1) * CPC] = np.asarray(outs[k], dtype=np.float32)
    return y


# revision 45
# speedup vs baseline: 1.4078x; 1.0274x over previous
"""Trainium2 Bass kernel for nn_ConvLogicLayer.

Computes y[n,c,oy,ox,p] = k0 + ka*A + kb*B + kab*A*B where A/B are
shifted-window gathers of input channels (per the packed `selection`),
and k* are per-(c,p) coefficients derived from softmax(weights) @ OP_COEFFS.

Strategy (v2, fp16):
  - Shard C_out (512) across 8 cores -> 64 output channels per core.
  - Specialized per-core program: gather indices and coefficients baked
    into the instruction stream (static APs + immediate scalars).
  - SBUF layout: partition q = n*4 + oyblk (32 images x 4 row-blocks),
    free dim = 64 input channels x 10 halo rows x 34 padded cols, fp16.
    A shifted 8x32 window for any (ch,ry,rx) is a static 3D AP.
  - fp16 everywhere: DVE runs tensor_scalar in 4x mode (127ns/[128,256])
    and tensor_tensor in 2x mode (194ns), and DMA bytes halve (in 5.6MB,
    out 16.8MB per core).  Host converts the fp16 output to f32.
  - Per pair (c,p): u = kab*B + ka (DVE ts), w = u*A (DVE tt).  Then
    either per-pair y = (kb*B + k0) + w via the AFFINE_THEN_ADD custom
    DVE uop (strided p-interleaved write, v folded), or per-channel
    merged y: v = kb*B + k0 (ACT/DVE/GPS) into v_all, then one GPSIMD
    scalar_tensor_tensor [128,1024] computes yc = w_all + v_all with the
    p-interleave expressed in the APs.  Mode chosen greedily per channel
    to balance DVE/ACT/GPS finish times (LP optimum ~89.8us/core).
  - y written p-interleaved so the per-channel output DMA (256KB fp16)
    is 2KB-contiguous per partition in HBM.  Input load is chunked and
    ordered by first use so compute overlaps the streaming load.
"""

import os
import sys
import threading

import numpy as np

for _p in ("/opt/trn_rl_repo",):
    if _p not in sys.path and os.path.isdir(_p):
        sys.path.insert(0, _p)

import concourse.bass as bass
import concourse.bacc as bacc
import concourse.mybir as mybir
from concourse.tile import TileContext
from concourse import bass_utils

# Problem constants (hardcoded per spec)
N, C_IN, H, W = 32, 64, 32, 32
C_OUT, KPAIRS = 512, 4
N_CORES = 8
CPC = C_OUT // N_CORES  # channels per core

P = 128          # partitions = (n=32) x (oyblk=4)
OYB = 4          # oy blocks per image
OYS = 8          # oy rows per block
HALO = 10        # rows stored per block (8 + 2 halo)
W34 = 34         # padded width
CHSZ = HALO * W34           # 340 elems per (q, channel)
XFREE = C_IN * CHSZ         # 21760 elems per partition
OUT_CSTRIDE = H * W * KPAIRS          # 4096
OUT_NSTRIDE = CPC * OUT_CSTRIDE       # 262144
SPP = OYS * W    # 256 elems per (partition, pair)

OP_COEFFS = np.array([
    [0.0, 0.0, 0.0, 0.0], [0.0, 0.0, 0.0, 1.0], [0.0, 1.0, 0.0, -1.0],
    [0.0, 1.0, 0.0, 0.0], [0.0, 0.0, 1.0, -1.0], [0.0, 0.0, 1.0, 0.0],
    [0.0, 1.0, 1.0, -2.0], [0.0, 1.0, 1.0, -1.0], [1.0, -1.0, -1.0, 1.0],
    [1.0, -1.0, -1.0, 2.0], [1.0, 0.0, -1.0, 0.0], [1.0, 0.0, -1.0, 1.0],
    [1.0, -1.0, 0.0, 0.0], [1.0, -1.0, 0.0, 1.0], [1.0, 0.0, 0.0, -1.0],
    [1.0, 0.0, 0.0, 0.0],
], dtype=np.float64)

MULT = mybir.AluOpType.mult
ADD = mybir.AluOpType.add
COPY = mybir.ActivationFunctionType.Copy
F16 = mybir.dt.float16

# TimelineSim per-op costs (fp16, [128,256] unless noted)
DVE_TS = 127.0    # tensor_scalar, 4x mode
DVE_TT = 194.0    # tensor_tensor contiguous, 2x mode
DVE_ATA = 327.0   # AFFINE_THEN_ADD custom uop (any stride)
ACT_TS = 398.0    # activation copy w/ scale+bias
GPS_TS = 451.0    # gpsimd tensor_scalar
GPS_TT_M = 2127.0   # gpsimd tensor_tensor [128,1024] merged y (stt is f32-only)
GPS_TT_S = 603.0    # gpsimd tensor_tensor [128,256] per-pair y
DVE_TT_M = 1127.0   # DVE tensor_tensor [128,1024] merged y (1x, strided in)

CFG = {
    "load_piece_ch": 4,  # channels per streamed load DMA
    "load_cascade": None,
    "tp_bufs": 8,    # u/w scratch tiles
    "wv_bufs": 5,    # w_all/v_all channel tiles (GPS merged-y inputs)
    "yc_bufs": 12,   # output channel tiles
    "pipe_depth": 0,  # channels of consumer delay (0 = inline)
    "head_ata": 0,   # first channels forced all-DVE
    "tail_ata": 0,   # last channels forced all-DVE
    "head_merged": 0,
    "gps_v_head": 0,  # first channels with v forced to GPSIMD
    "gps_single": 0,  # per-pair GPS y instead of per-channel merged
    "gps_half": 0,    # split merged y into two half-channel ops
    "auto_quota": 0,
    "f_ata": 0.28,    # fraction of channels on the all-DVE ATA path
    "uv_act": 0.445,  # fraction of merged-channel u/v ops on ACT
}

last_results = [None] * N_CORES  # BassKernelResults per core (for profiling)
last_model_ns = [None] * N_CORES  # per-core TimelineSim estimate
last_eng_ns = [None] * N_CORES  # per-core greedy engine-load projection


def core_channel_perm(core, ch):
    """Input-channel permutation for `core`: most-used first.

    Returns (perm, inv) with perm[pos] = original channel stored at slot pos
    in this core's xh, inv[orig] = slot.
    """
    use_cnt = [0] * C_IN
    for cl in range(CPC):
        for j in range(2 * KPAIRS):
            use_cnt[int(ch[core * CPC + cl, j])] += 1
    perm = sorted(range(C_IN), key=lambda i: (-use_cnt[i], i))
    inv = [0] * C_IN
    for pos, orig in enumerate(perm):
        inv[orig] = pos
    return perm, inv


def build_core_program(core, ch, ry, rx, coef):
    """One specialized Bass program for `core` (channels core*CPC..+CPC).

    The host stores this core's xh with channels permuted most-used-first
    (core_channel_perm); `ch` is remapped here to slot indices so the
    cascade [s0, s1, ...] loads contiguous slabs front-to-back.
    """
    nc = bacc.Bacc("TRN2", target_bir_lowering=False)
    xh_d = nc.dram_tensor("xh", [P, XFREE], F16, kind="ExternalInput")
    out_d = nc.dram_tensor(
        "out", [N, CPC, H, W, KPAIRS], F16, kind="ExternalOutput"
    )

    eng_ns = {
        "dve": float(CFG.get("seed_dve", 0.0)),
        "act": float(CFG.get("seed_act", 0.0)),
        "gps": float(CFG.get("seed_gps", 0.0)),
    }

    with TileContext(nc) as tc:
        with (
            tc.tile_pool(name="xp", bufs=1) as xpool,
            tc.tile_pool(name="tp", bufs=CFG["tp_bufs"]) as tpool,
            tc.tile_pool(name="wv", bufs=CFG["wv_bufs"]) as wvpool,
            tc.tile_pool(name="yp", bufs=CFG["yc_bufs"]) as ypool,
        ):
            xh = xpool.tile([P, XFREE], F16)
            # ch remapped to this core's permuted slots (most-used first).
            _, inv = core_channel_perm(core, ch)
            chv = {
                (cl, j): inv[int(ch[core * CPC + cl, j])]
                for cl in range(CPC)
                for j in range(2 * KPAIRS)
            }
            # Streamed load: many small contiguous piece-DMAs front-to-back
            # (most-used slots first).  Pairs/channels are ordered by the
            # piece at which both operands are resident, so the in-order
            # engine queues never head-of-line block on a late channel.
            sizes = CFG.get("load_cascade")
            if not sizes:
                g = CFG.get("load_piece_ch", 4)
                sizes = [g] * (C_IN // g) + ([C_IN % g] if C_IN % g else [])
            bounds = []
            acc = 0
            for sz in sizes:
                acc += sz
                bounds.append(acc)
            assert bounds[-1] == C_IN, bounds

            def tier_of_slot(slot):
                for t, b in enumerate(bounds):
                    if slot < b:
                        return t
                raise AssertionError(slot)

            ptier = {
                (cl, p4): max(
                    tier_of_slot(chv[(cl, 2 * p4)]), tier_of_slot(chv[(cl, 2 * p4 + 1)])
                )
                for cl in range(CPC)
                for p4 in range(KPAIRS)
            }
            tiers = {cl: sorted(ptier[(cl, p4)] for p4 in range(KPAIRS)) for cl in range(CPC)}
            # sort by completion tier (max first): a channel is consumable
            # only once its LAST pair's inputs arrive, so straggler channels
            # must not sit at the front of the in-order queues.
            cl_order = sorted(range(CPC), key=lambda cl: tiers[cl][::-1])
            p4_order = {
                cl: sorted(range(KPAIRS), key=lambda p4: ptier[(cl, p4)])
                for cl in range(CPC)
            }
            lo = 0
            for b in bounds:
                nc.sync.dma_start(
                    xh[:, lo * CHSZ : b * CHSZ], xh_d[:, lo * CHSZ : b * CHSZ]
                )
                lo = b
            base = xh[:]
            pitch = base.ap[0][0]
            tens = base.tensor
            base_off = base.offset

            # Software pipelining: emit each channel's producers (u/w/v) now
            # but its consumers (merged gstt + output DMA) D channels later,
            # so no engine's queue head waits on a just-issued cross-engine
            # dependency.
            pipe_d = CFG.get("pipe_depth", 3)
            pending = []  # (w_all, v_all, yc, oap, y_eng) awaiting merge+DMA
            st = {"mp": 0, "ua": 0, "va": 0}  # merged-pair / ACT-quota counters

            def flush_one():
                w_all, v_all, yc_t, oap_t, y_eng = pending.pop(0)
                yb = yc_t[:]
                yp_ = yb.ap[0][0]
                wb = w_all[:]
                vb = v_all[:]
                halves = CFG.get("gps_half", 0)
                nh = 2 if halves else 1
                pk = KPAIRS // nh
                for h in range(nh):
                    w_ap = bass.AP(wb.tensor, wb.offset + h * pk * SPP,
                                   [[wb.ap[0][0], P], [1, SPP], [SPP, pk]])
                    v_ap = bass.AP(vb.tensor, vb.offset + h * pk * SPP,
                                   [[vb.ap[0][0], P], [1, SPP], [SPP, pk]])
                    y_ap = bass.AP(yb.tensor, yb.offset + h * pk,
                                   [[yp_, P], [KPAIRS, SPP], [1, pk]])
                    if y_eng == "gps":
                        nc.gpsimd.tensor_tensor(y_ap, w_ap, v_ap, ADD)
                    else:
                        nc.vector.tensor_tensor(y_ap, w_ap, v_ap, ADD)
                nc.sync.dma_start(oap_t, yb)

            for cl in cl_order:
                c = core * CPC + cl
                yc = ypool.tile([P, OYS * W * KPAIRS], F16, tag="yc")
                ybase = yc[:]
                ypitch = ybase.ap[0][0]

                # channel mode: merged y on GPS vs per-pair ATA on DVE.
                # Project both and keep whichever minimizes the makespan.
                # LP-quota assignment: mode basis is m3 (u ACT + all-DVE ATA),
                # m5 (u/v DVE + y GPS-merged), m7 (u/v ACT + y GPS-merged).
                # Solved so each engine's start offset + load is equal --
                # engines finish together rather than having equal totals.
                pos = cl_order.index(cl)
                f_ata = CFG.get("f_ata", 0.281)
                uv_act = CFG.get("uv_act", 0.473)
                if CFG.get("auto_quota", 0):
                    M = np.array([
                        [521.0, 448.0, 194.0, -1.0],
                        [398.0, 0.0, 796.0, -1.0],
                        [0.0, 532.0, 532.0, -1.0],
                        [1.0, 1.0, 1.0, 0.0],
                    ])
                    rhs = np.array([
                        -float(CFG.get("off_dve", 6500.0)),
                        -float(CFG.get("off_act", 6500.0)),
                        -float(CFG.get("off_gps", 11000.0)),
                        float(C_OUT // N_CORES * KPAIRS),
                    ])
                    x3, x5, x7, _T = np.linalg.solve(M, rhs)
                    x3, x5, x7 = max(x3, 0.0), max(x5, 0.0), max(x7, 0.0)
                    tot = x3 + x5 + x7
                    f_ata = x3 / tot
                    uv_act = x7 / max(x5 + x7, 1e-9)
                n_ata = int(round(f_ata * CPC))
                tail = min(CFG.get("tail_ata", 0), n_ata)
                head_m = CFG.get("head_merged", 0)
                spread = n_ata - tail
                span = CPC - tail - head_m
                if pos >= CPC - tail:
                    is_ata = True
                elif pos < head_m or span <= 0:
                    is_ata = False
                else:
                    q = pos - head_m
                    is_ata = int(q * spread / span) < int((q + 1) * spread / span)
                if pos < CFG.get("head_ata", 0):
                    is_ata = True
                if is_ata:
                    mode = "ata"
                elif CFG.get("gps_single", 0):
                    mode = "gps_s"  # per-pair gtt, strided out (no w_all/v_all)
                else:
                    mode = "gps_m"
                merged = mode == "gps_m"
                uplace, vplace = [], []
                for _ in range(KPAIRS):
                    if is_ata:
                        uplace.append("act")
                        eng_ns["act"] += ACT_TS
                        eng_ns["dve"] += DVE_TT + DVE_ATA
                        continue
                    if mode == "gps_s":
                        eng_ns["gps"] += GPS_TT_S
                    st["mp"] += 1
                    # u placement against the ACT quota
                    if st["ua"] < uv_act * st["mp"]:
                        uplace.append("act")
                        st["ua"] += 1
                        eng_ns["act"] += ACT_TS
                    else:
                        uplace.append("dve")
                        eng_ns["dve"] += DVE_TS
                    eng_ns["dve"] += DVE_TT
                    if pos < CFG.get("gps_v_head", 0):
                        vplace.append("gps")
                        eng_ns["gps"] += GPS_TS
                    elif st["va"] < uv_act * st["mp"]:
                        vplace.append("act")
                        st["va"] += 1
                        eng_ns["act"] += ACT_TS
                    else:
                        vplace.append("dve")
                        eng_ns["dve"] += DVE_TS
                if merged:
                    eng_ns["gps"] += 2222.0 if CFG.get("gps_half", 0) else GPS_TT_M
                    w_all = wvpool.tile([P, KPAIRS * SPP], F16, tag="wa")
                    v_all = wvpool.tile([P, KPAIRS * SPP], F16, tag="va")

                for i, p4 in enumerate(p4_order[cl]):
                    ka_, kb_ = 2 * p4, 2 * p4 + 1
                    offA = base_off + chv[(cl, ka_)] * CHSZ + int(ry[c, ka_]) * W34 + int(rx[c, ka_])
                    offB = base_off + chv[(cl, kb_)] * CHSZ + int(ry[c, kb_]) * W34 + int(rx[c, kb_])
                    A_ap = bass.AP(tens, offA, [[pitch, P], [W34, OYS], [1, W]])
                    B_ap = bass.AP(tens, offB, [[pitch, P], [W34, OYS], [1, W]])

                    k0 = float(coef[c, p4, 0])
                    ka = float(coef[c, p4, 1])
                    kb = float(coef[c, p4, 2])
                    kab = float(coef[c, p4, 3])

                    u = tpool.tile([P, SPP], F16, tag="u")
                    u3 = u[:].rearrange("p (a b) -> p a b", b=W)
                    # u = kab*B + ka
                    ue = uplace[i]
                    if ue == "act":
                        nc.scalar.activation(u3, B_ap, COPY, bias=ka, scale=kab)
                    elif ue == "gps":
                        nc.gpsimd.tensor_scalar(u3, B_ap, kab, ka, MULT, ADD)
                    else:
                        nc.vector.tensor_scalar(u3, B_ap, kab, ka, MULT, ADD)

                    if merged:
                        wsl = w_all[:, p4 * SPP : (p4 + 1) * SPP]
                        w3 = wsl.rearrange("p (a b) -> p a b", b=W)
                        # w = u*A  (DVE tt, 2x, contiguous out)
                        nc.vector.tensor_tensor(w3, u3, A_ap, MULT)
                        vsl = v_all[:, p4 * SPP : (p4 + 1) * SPP]
                        v3 = vsl.rearrange("p (a b) -> p a b", b=W)
                        e = vplace[i]
                        if e == "act":
                            nc.scalar.activation(v3, B_ap, COPY, bias=k0, scale=kb)
                        elif e == "gps":
                            nc.gpsimd.tensor_scalar(v3, B_ap, kb, k0, MULT, ADD)
                        else:
                            nc.vector.tensor_scalar(v3, B_ap, kb, k0, MULT, ADD)
                    elif mode == "gps_s":
                        w = tpool.tile([P, SPP], F16, tag="w")
                        v = tpool.tile([P, SPP], F16, tag="v")
                        w3 = w[:].rearrange("p (a b) -> p a b", b=W)
                        v3 = v[:].rearrange("p (a b) -> p a b", b=W)
                        nc.vector.tensor_tensor(w3, u3, A_ap, MULT)
                        e = vplace[i]
                        if e == "act":
                            nc.scalar.activation(v3, B_ap, COPY, bias=k0, scale=kb)
                        elif e == "gps":
                            nc.gpsimd.tensor_scalar(v3, B_ap, kb, k0, MULT, ADD)
                        else:
                            nc.vector.tensor_scalar(v3, B_ap, kb, k0, MULT, ADD)
                        yap = bass.AP(
                            ybase.tensor, ybase.offset + p4,
                            [[ypitch, P], [W * KPAIRS, OYS], [KPAIRS, W]],
                        )
                        nc.gpsimd.tensor_tensor(yap, w3, v3, ADD)
                    else:
                        w = tpool.tile([P, SPP], F16, tag="w")
                        w3 = w[:].rearrange("p (a b) -> p a b", b=W)
                        nc.vector.tensor_tensor(w3, u3, A_ap, MULT)
                        # y = (kb*B + k0) + w, p-interleaved into yc (custom uop)
                        yap = bass.AP(
                            ybase.tensor, ybase.offset + p4,
                            [[ypitch, P], [W * KPAIRS, OYS], [KPAIRS, W]],
                        )
                        nc.vector.affine_then_add(yap, B_ap, w3, kb, k0)

                # HBM [n, oyblk, (oy',ox,p)=1024]
                oap = bass.AP(
                    out_d, cl * OUT_CSTRIDE,
                    [[OUT_NSTRIDE, N], [OYS * W * KPAIRS, OYB], [1, OYS * W * KPAIRS]],
                )
                if merged:
                    pending.append((w_all, v_all, yc, oap, "gps" if mode == "gps_m" else "dve"))
                    if len(pending) > pipe_d:
                        flush_one()
                else:
                    nc.sync.dma_start(oap, ybase)
            while pending:
                flush_one()
    nc.finalize()  # Bacc: splits >1-wait syncs into event semaphores
    last_eng_ns[core] = dict(eng_ns)
    return nc


def _prep_inputs(x, weights, selection):
    x = np.ascontiguousarray(np.asarray(x, dtype=np.float32))
    weights = np.asarray(weights, dtype=np.float32)
    selection = np.asarray(selection, dtype=np.int32)

    # coefficients: softmax over 16 logic ops folded into {1,a,b,ab} basis
    w64 = weights.astype(np.float64)
    e = np.exp(w64 - w64.max(axis=-1, keepdims=True))
    prob = e / e.sum(axis=-1, keepdims=True)
    coef = (prob @ OP_COEFFS).astype(np.float32)  # [C_OUT, 4, 4]

    ch = ((selection >> 16) & 0xFFFF).astype(np.int64)
    ry = ((selection >> 8) & 0xFF).astype(np.int64)
    rx = (selection & 0xFF).astype(np.int64)

    # halo layout: xh[q=(n,oyblk), ch, r, w] = xpad[n, ch, oyblk*8+r, w]
    xpad = np.zeros((N, C_IN, H + 2, W + 2), dtype=np.float32)
    xpad[:, :, 1 : H + 1, 1 : W + 1] = x
    xh = np.empty((N, OYB, C_IN, HALO, W34), dtype=np.float16)
    for b in range(OYB):
        xh[:, b] = xpad[:, :, b * OYS : b * OYS + HALO, :]
    # per-core copies with channels permuted most-used-first so the load
    # cascade is a few big contiguous DMAs
    xh_cores = []
    for k in range(N_CORES):
        perm, _ = core_channel_perm(k, ch)
        xh_cores.append(np.ascontiguousarray(xh[:, :, perm].reshape(P, XFREE)))
    return xh_cores, ch, ry, rx, coef


def kernel(x, weights, selection):
    assert x.shape == (N, C_IN, H, W), x.shape
    assert weights.shape == (C_OUT, 4, 16), weights.shape
    assert selection.shape == (C_OUT, 8), selection.shape

    xh_cores, ch, ry, rx, coef = _prep_inputs(x, weights, selection)

    try:
        from concourse.timeline_sim import TimelineSim
    except Exception:  # noqa: BLE001
        TimelineSim = None
    # Per-core auto-tune: each core's selection pattern favors different
    # schedule knobs; build candidates and keep the TimelineSim-fastest.
    cands = CFG.get(
        "tune_candidates",
        (
            {},
            {"load_piece_ch": 6},
            {"f_ata": 0.26},
            {"f_ata": 0.31},
            {"uv_act": 0.40},
            {"uv_act": 0.50},
        ),
    )
    progs = []
    base_cfg = dict(CFG)
    for k in range(N_CORES):
        best = None
        for cfg_delta in cands if TimelineSim is not None else ({},):
            CFG.clear()
            CFG.update(base_cfg)
            CFG.update(cfg_delta)
            nc = build_core_program(k, ch, ry, rx, coef)
            ns = None
            if TimelineSim is not None:
                try:
                    ns = TimelineSim(nc, trace=False).simulate()
                except Exception:  # noqa: BLE001
                    ns = None
            if best is None or (ns is not None and best[0] is not None and ns < best[0]):
                best = (ns, nc)
            if ns is None:
                break
        progs.append(best[1])
        last_model_ns[k] = best[0]
    CFG.clear()
    CFG.update(base_cfg)

    import jax

    devices = jax.devices()
    assert len(devices) >= N_CORES, devices

    outs = [None] * N_CORES
    errs = [None] * N_CORES
    # NTFF tracing needs axon hooks that aren't present in this container —
    # make sure run_bass_kernel_spmd never tries (BASS_TRACE in env would).
    os.environ["BASS_NEVER_TRACE"] = "1"

    def run_one(k):
        try:
            with jax.default_device(devices[k]):
                res = bass_utils.run_bass_kernel_spmd(
                    progs[k], [{"xh": xh_cores[k]}], core_ids=[k]
                )
            last_results[k] = res
            outs[k] = res.results[0]["out"]
        except Exception as e:  # noqa: BLE001
            errs[k] = e

    threads = [threading.Thread(target=run_one, args=(k,)) for k in range(N_CORES)]
    for t in threads:
        t.start()
    for t in threads:
        t.join()
    for k, e in enumerate(errs):
        if e is not None:
            raise RuntimeError(f"core {k} failed") from e

    y = np.empty((N, C_OUT, H, W, KPAIRS), dtype=np.float32)
    for k in range(N_CORES):
        y[:, k * CPC : (k + 1) * CPC] = np.asarray(outs[k], dtype=np.float32)
    return y


# revision 49
# speedup vs baseline: 1.4445x; 1.0260x over previous
"""Trainium2 Bass kernel for nn_ConvLogicLayer.

Computes y[n,c,oy,ox,p] = k0 + ka*A + kb*B + kab*A*B where A/B are
shifted-window gathers of input channels (per the packed `selection`),
and k* are per-(c,p) coefficients derived from softmax(weights) @ OP_COEFFS.

Strategy (v2, fp16, multi-engine LP balance):
  - Shard C_out (512) across 8 cores -> 64 output channels per core.
  - Specialized per-core program: gather indices and coefficients baked
    into the instruction stream (static APs + immediate scalars).
  - SBUF layout: partition q = n*4 + oyblk (32 images x 4 row-blocks),
    free dim = 64 input channels x 10 halo rows x 34 padded cols, fp16,
    channels permuted per-core most-used-first.  A shifted 8x32 window
    for any (ch,ry,rx) is a static 3D AP.  The load streams as small
    contiguous piece-DMAs; channels are processed in completion-tier
    order so the in-order engine queues never wait on a late channel.
  - fp16 everywhere: DVE tensor_scalar runs in 4x mode (127ns/[128,256]),
    tensor_tensor in 2x mode (194ns), DMA bytes halve (in 5.6MB, out
    16.8MB per core).  Host converts the fp16 output to f32; rel err vs
    the f32 reference is ~1e-3 (tolerance 2e-2).
  - Per pair (c,p): u = kab*B + ka (DVE ts or ACT activation), then
    w = u*A (DVE tt).  y two ways, split by an offline LP over the
    per-op engine costs (DVE/ACT/GPSIMD finish together, ~98us loads):
      * f_ata of channels: per-pair y = (kb*B + k0) + w via the
        AFFINE_THEN_ADD custom DVE uop (strided p-interleaved write,
        v folded free; u on ACT for these channels);
      * the rest: v = kb*B + k0 (uv_act of u/v on ACT, rest DVE) into
        v_all, then ONE GPSIMD tensor_tensor [128,1024] per channel
        computes yc = w_all + v_all with the p-interleave expressed in
        the APs.  (scalar_tensor_tensor on Pool is f32-only in the real
        backend -- tensor_tensor ADD is the fp16-legal form.)
  - y written p-interleaved so the per-channel output DMA (256KB fp16)
    is 2KB-contiguous per partition in HBM and overlaps compute.
  - Per-core auto-tune over schedule knobs via TimelineSim.
"""

import os
import sys
import threading

import numpy as np

for _p in ("/opt/trn_rl_repo",):
    if _p not in sys.path and os.path.isdir(_p):
        sys.path.insert(0, _p)

import concourse.bass as bass
import concourse.bacc as bacc
import concourse.mybir as mybir
from concourse.tile import TileContext
from concourse import bass_utils

# Problem constants (hardcoded per spec)
N, C_IN, H, W = 32, 64, 32, 32
C_OUT, KPAIRS = 512, 4
N_CORES = 8
CPC = C_OUT // N_CORES  # channels per core

P = 128          # partitions = (n=32) x (oyblk=4)
OYB = 4          # oy blocks per image
OYS = 8          # oy rows per block
HALO = 10        # rows stored per block (8 + 2 halo)
W34 = 34         # padded width
CHSZ = HALO * W34           # 340 elems per (q, channel)
XFREE = C_IN * CHSZ         # 21760 elems per partition
OUT_CSTRIDE = H * W * KPAIRS          # 4096
OUT_NSTRIDE = CPC * OUT_CSTRIDE       # 262144
SPP = OYS * W    # 256 elems per (partition, pair)

OP_COEFFS = np.array([
    [0.0, 0.0, 0.0, 0.0], [0.0, 0.0, 0.0, 1.0], [0.0, 1.0, 0.0, -1.0],
    [0.0, 1.0, 0.0, 0.0], [0.0, 0.0, 1.0, -1.0], [0.0, 0.0, 1.0, 0.0],
    [0.0, 1.0, 1.0, -2.0], [0.0, 1.0, 1.0, -1.0], [1.0, -1.0, -1.0, 1.0],
    [1.0, -1.0, -1.0, 2.0], [1.0, 0.0, -1.0, 0.0], [1.0, 0.0, -1.0, 1.0],
    [1.0, -1.0, 0.0, 0.0], [1.0, -1.0, 0.0, 1.0], [1.0, 0.0, 0.0, -1.0],
    [1.0, 0.0, 0.0, 0.0],
], dtype=np.float64)

MULT = mybir.AluOpType.mult
ADD = mybir.AluOpType.add
COPY = mybir.ActivationFunctionType.Copy
F16 = mybir.dt.float16

# TimelineSim per-op costs (fp16, [128,256] unless noted)
DVE_TS = 127.0    # tensor_scalar, 4x mode
DVE_TT = 194.0    # tensor_tensor contiguous, 2x mode
DVE_ATA = 327.0   # AFFINE_THEN_ADD custom uop (any stride)
ACT_TS = 398.0    # activation copy w/ scale+bias
GPS_TS = 451.0    # gpsimd tensor_scalar
GPS_TT_M = 2127.0   # gpsimd tensor_tensor [128,1024] merged y (stt is f32-only)
GPS_TT_S = 603.0    # gpsimd tensor_tensor [128,256] per-pair y
DVE_TT_M = 1127.0   # DVE tensor_tensor [128,1024] merged y (1x, strided in)

CFG = {
    "load_piece_ch": 4,  # channels per streamed load DMA
    "load_cascade": None,
    "tp_bufs": 8,    # u/w scratch tiles
    "wv_bufs": 5,    # w_all/v_all channel tiles (GPS merged-y inputs)
    "yc_bufs": 12,   # output channel tiles
    "pipe_depth": 0,  # channels of consumer delay (0 = inline)
    "head_ata": 0,   # first channels forced all-DVE
    "tail_ata": 0,   # last channels forced all-DVE
    "head_merged": 0,
    "gps_v_head": 0,  # first channels with v forced to GPSIMD
    "gps_single": 0,  # per-pair GPS y instead of per-channel merged
    "gps_half": 0,    # split merged y into two half-channel ops
    "auto_quota": 0,
    "f_ata": 0.28,    # fraction of channels on the all-DVE ATA path
    "uv_act": 0.445,  # fraction of merged-channel u/v ops on ACT
}

last_results = [None] * N_CORES  # BassKernelResults per core (for profiling)
last_model_ns = [None] * N_CORES  # per-core TimelineSim estimate
last_eng_ns = [None] * N_CORES  # per-core greedy engine-load projection


def core_channel_perm(core, ch):
    """Input-channel permutation for `core`: most-used first.

    Returns (perm, inv) with perm[pos] = original channel stored at slot pos
    in this core's xh, inv[orig] = slot.
    """
    use_cnt = [0] * C_IN
    for cl in range(CPC):
        for j in range(2 * KPAIRS):
            use_cnt[int(ch[core * CPC + cl, j])] += 1
    perm = sorted(range(C_IN), key=lambda i: (-use_cnt[i], i))
    inv = [0] * C_IN
    for pos, orig in enumerate(perm):
        inv[orig] = pos
    return perm, inv


def build_core_program(core, ch, ry, rx, coef):
    """One specialized Bass program for `core` (channels core*CPC..+CPC).

    The host stores this core's xh with channels permuted most-used-first
    (core_channel_perm); `ch` is remapped here to slot indices so the
    cascade [s0, s1, ...] loads contiguous slabs front-to-back.
    """
    nc = bacc.Bacc("TRN2", target_bir_lowering=False)
    xh_d = nc.dram_tensor("xh", [P, XFREE], F16, kind="ExternalInput")
    out_d = nc.dram_tensor(
        "out", [N, CPC, H, W, KPAIRS], F16, kind="ExternalOutput"
    )

    eng_ns = {
        "dve": float(CFG.get("seed_dve", 0.0)),
        "act": float(CFG.get("seed_act", 0.0)),
        "gps": float(CFG.get("seed_gps", 0.0)),
    }

    with TileContext(nc) as tc:
        with (
            tc.tile_pool(name="xp", bufs=1) as xpool,
            tc.tile_pool(name="tp", bufs=CFG["tp_bufs"]) as tpool,
            tc.tile_pool(name="wv", bufs=CFG["wv_bufs"]) as wvpool,
            tc.tile_pool(name="yp", bufs=CFG["yc_bufs"]) as ypool,
        ):
            xh = xpool.tile([P, XFREE], F16)
            # ch remapped to this core's permuted slots (most-used first).
            _, inv = core_channel_perm(core, ch)
            chv = {
                (cl, j): inv[int(ch[core * CPC + cl, j])]
                for cl in range(CPC)
                for j in range(2 * KPAIRS)
            }
            # Streamed load: many small contiguous piece-DMAs front-to-back
            # (most-used slots first).  Pairs/channels are ordered by the
            # piece at which both operands are resident, so the in-order
            # engine queues never head-of-line block on a late channel.
            sizes = CFG.get("load_cascade")
            if not sizes:
                g = CFG.get("load_piece_ch", 4)
                sizes = [g] * (C_IN // g) + ([C_IN % g] if C_IN % g else [])
            bounds = []
            acc = 0
            for sz in sizes:
                acc += sz
                bounds.append(acc)
            assert bounds[-1] == C_IN, bounds

            def tier_of_slot(slot):
                for t, b in enumerate(bounds):
                    if slot < b:
                        return t
                raise AssertionError(slot)

            ptier = {
                (cl, p4): max(
                    tier_of_slot(chv[(cl, 2 * p4)]), tier_of_slot(chv[(cl, 2 * p4 + 1)])
                )
                for cl in range(CPC)
                for p4 in range(KPAIRS)
            }
            tiers = {cl: sorted(ptier[(cl, p4)] for p4 in range(KPAIRS)) for cl in range(CPC)}
            # sort by completion tier (max first): a channel is consumable
            # only once its LAST pair's inputs arrive, so straggler channels
            # must not sit at the front of the in-order queues.
            cl_order = sorted(range(CPC), key=lambda cl: tiers[cl][::-1])
            p4_order = {
                cl: sorted(range(KPAIRS), key=lambda p4: ptier[(cl, p4)])
                for cl in range(CPC)
            }
            lo = 0
            for b in bounds:
                nc.sync.dma_start(
                    xh[:, lo * CHSZ : b * CHSZ], xh_d[:, lo * CHSZ : b * CHSZ]
                )
                lo = b
            base = xh[:]
            pitch = base.ap[0][0]
            tens = base.tensor
            base_off = base.offset

            # Software pipelining: emit each channel's producers (u/w/v) now
            # but its consumers (merged gstt + output DMA) D channels later,
            # so no engine's queue head waits on a just-issued cross-engine
            # dependency.
            pipe_d = CFG.get("pipe_depth", 3)
            pending = []  # (w_all, v_all, yc, oap, y_eng) awaiting merge+DMA
            st = {"mp": 0, "ua": 0, "va": 0}  # merged-pair / ACT-quota counters

            def flush_one():
                w_all, v_all, yc_t, oap_t, y_eng = pending.pop(0)
                yb = yc_t[:]
                yp_ = yb.ap[0][0]
                wb = w_all[:]
                vb = v_all[:]
                halves = CFG.get("gps_half", 0)
                nh = 2 if halves else 1
                pk = KPAIRS // nh
                for h in range(nh):
                    w_ap = bass.AP(wb.tensor, wb.offset + h * pk * SPP,
                                   [[wb.ap[0][0], P], [1, SPP], [SPP, pk]])
                    v_ap = bass.AP(vb.tensor, vb.offset + h * pk * SPP,
                                   [[vb.ap[0][0], P], [1, SPP], [SPP, pk]])
                    y_ap = bass.AP(yb.tensor, yb.offset + h * pk,
                                   [[yp_, P], [KPAIRS, SPP], [1, pk]])
                    if y_eng == "gps":
                        nc.gpsimd.tensor_tensor(y_ap, w_ap, v_ap, ADD)
                    else:
                        nc.vector.tensor_tensor(y_ap, w_ap, v_ap, ADD)
                nc.sync.dma_start(oap_t, yb)

            for cl in cl_order:
                c = core * CPC + cl
                yc = ypool.tile([P, OYS * W * KPAIRS], F16, tag="yc")
                ybase = yc[:]
                ypitch = ybase.ap[0][0]

                # channel mode: merged y on GPS vs per-pair ATA on DVE.
                # Project both and keep whichever minimizes the makespan.
                # LP-quota assignment: mode basis is m3 (u ACT + all-DVE ATA),
                # m5 (u/v DVE + y GPS-merged), m7 (u/v ACT + y GPS-merged).
                # Solved so each engine's start offset + load is equal --
                # engines finish together rather than having equal totals.
                pos = cl_order.index(cl)
                f_ata = CFG.get("f_ata", 0.281)
                uv_act = CFG.get("uv_act", 0.473)
                if CFG.get("auto_quota", 0):
                    M = np.array([
                        [521.0, 448.0, 194.0, -1.0],
                        [398.0, 0.0, 796.0, -1.0],
                        [0.0, 532.0, 532.0, -1.0],
                        [1.0, 1.0, 1.0, 0.0],
                    ])
                    rhs = np.array([
                        -float(CFG.get("off_dve", 6500.0)),
                        -float(CFG.get("off_act", 6500.0)),
                        -float(CFG.get("off_gps", 11000.0)),
                        float(C_OUT // N_CORES * KPAIRS),
                    ])
                    x3, x5, x7, _T = np.linalg.solve(M, rhs)
                    x3, x5, x7 = max(x3, 0.0), max(x5, 0.0), max(x7, 0.0)
                    tot = x3 + x5 + x7
                    f_ata = x3 / tot
                    uv_act = x7 / max(x5 + x7, 1e-9)
                n_ata = int(round(f_ata * CPC))
                tail = min(CFG.get("tail_ata", 0), n_ata)
                head_m = CFG.get("head_merged", 0)
                spread = n_ata - tail
                span = CPC - tail - head_m
                if pos >= CPC - tail:
                    is_ata = True
                elif pos < head_m or span <= 0:
                    is_ata = False
                else:
                    q = pos - head_m
                    is_ata = int(q * spread / span) < int((q + 1) * spread / span)
                if pos < CFG.get("head_ata", 0):
                    is_ata = True
                if is_ata:
                    mode = "ata"
                elif pos < CFG.get("dvem_head", 0):
                    # head channels' merged y on DVE: GPS is still ramping in,
                    # so this trims GPS's total without delaying its start.
                    mode = "dve_m"
                elif CFG.get("gps_single", 0):
                    mode = "gps_s"  # per-pair gtt, strided out (no w_all/v_all)
                else:
                    mode = "gps_m"
                merged = mode in ("gps_m", "dve_m")
                uplace, vplace = [], []
                for _ in range(KPAIRS):
                    if is_ata:
                        uplace.append("act")
                        eng_ns["act"] += ACT_TS
                        eng_ns["dve"] += DVE_TT + DVE_ATA
                        continue
                    if mode == "gps_s":
                        eng_ns["gps"] += GPS_TT_S
                    st["mp"] += 1
                    # u placement against the ACT quota
                    if st["ua"] < uv_act * st["mp"]:
                        uplace.append("act")
                        st["ua"] += 1
                        eng_ns["act"] += ACT_TS
                    else:
                        uplace.append("dve")
                        eng_ns["dve"] += DVE_TS
                    eng_ns["dve"] += DVE_TT
                    if pos < CFG.get("gps_v_head", 0):
                        vplace.append("gps")
                        eng_ns["gps"] += GPS_TS
                    elif st["va"] < uv_act * st["mp"]:
                        vplace.append("act")
                        st["va"] += 1
                        eng_ns["act"] += ACT_TS
                    else:
                        vplace.append("dve")
                        eng_ns["dve"] += DVE_TS
                if merged:
                    if mode == "dve_m":
                        eng_ns["dve"] += DVE_TT_M
                    else:
                        eng_ns["gps"] += 2222.0 if CFG.get("gps_half", 0) else GPS_TT_M
                    w_all = wvpool.tile([P, KPAIRS * SPP], F16, tag="wa")
                    v_all = wvpool.tile([P, KPAIRS * SPP], F16, tag="va")

                for i, p4 in enumerate(p4_order[cl]):
                    ka_, kb_ = 2 * p4, 2 * p4 + 1
                    offA = base_off + chv[(cl, ka_)] * CHSZ + int(ry[c, ka_]) * W34 + int(rx[c, ka_])
                    offB = base_off + chv[(cl, kb_)] * CHSZ + int(ry[c, kb_]) * W34 + int(rx[c, kb_])
                    A_ap = bass.AP(tens, offA, [[pitch, P], [W34, OYS], [1, W]])
                    B_ap = bass.AP(tens, offB, [[pitch, P], [W34, OYS], [1, W]])

                    k0 = float(coef[c, p4, 0])
                    ka = float(coef[c, p4, 1])
                    kb = float(coef[c, p4, 2])
                    kab = float(coef[c, p4, 3])

                    u = tpool.tile([P, SPP], F16, tag="u")
                    u3 = u[:].rearrange("p (a b) -> p a b", b=W)
                    # u = kab*B + ka
                    ue = uplace[i]
                    if ue == "act":
                        nc.scalar.activation(u3, B_ap, COPY, bias=ka, scale=kab)
                    elif ue == "gps":
                        nc.gpsimd.tensor_scalar(u3, B_ap, kab, ka, MULT, ADD)
                    else:
                        nc.vector.tensor_scalar(u3, B_ap, kab, ka, MULT, ADD)

                    if merged:
                        wsl = w_all[:, p4 * SPP : (p4 + 1) * SPP]
                        w3 = wsl.rearrange("p (a b) -> p a b", b=W)
                        # w = u*A  (DVE tt, 2x, contiguous out)
                        nc.vector.tensor_tensor(w3, u3, A_ap, MULT)
                        vsl = v_all[:, p4 * SPP : (p4 + 1) * SPP]
                        v3 = vsl.rearrange("p (a b) -> p a b", b=W)
                        e = vplace[i]
                        if e == "act":
                            nc.scalar.activation(v3, B_ap, COPY, bias=k0, scale=kb)
                        elif e == "gps":
                            nc.gpsimd.tensor_scalar(v3, B_ap, kb, k0, MULT, ADD)
                        else:
                            nc.vector.tensor_scalar(v3, B_ap, kb, k0, MULT, ADD)
                    elif mode == "gps_s":
                        w = tpool.tile([P, SPP], F16, tag="w")
                        v = tpool.tile([P, SPP], F16, tag="v")
                        w3 = w[:].rearrange("p (a b) -> p a b", b=W)
                        v3 = v[:].rearrange("p (a b) -> p a b", b=W)
                        nc.vector.tensor_tensor(w3, u3, A_ap, MULT)
                        e = vplace[i]
                        if e == "act":
                            nc.scalar.activation(v3, B_ap, COPY, bias=k0, scale=kb)
                        elif e == "gps":
                            nc.gpsimd.tensor_scalar(v3, B_ap, kb, k0, MULT, ADD)
                        else:
                            nc.vector.tensor_scalar(v3, B_ap, kb, k0, MULT, ADD)
                        yap = bass.AP(
                            ybase.tensor, ybase.offset + p4,
                            [[ypitch, P], [W * KPAIRS, OYS], [KPAIRS, W]],
                        )
                        nc.gpsimd.tensor_tensor(yap, w3, v3, ADD)
                    else:
                        w = tpool.tile([P, SPP], F16, tag="w")
                        w3 = w[:].rearrange("p (a b) -> p a b", b=W)
                        nc.vector.tensor_tensor(w3, u3, A_ap, MULT)
                        # y = (kb*B + k0) + w, p-interleaved into yc (custom uop)
                        yap = bass.AP(
                            ybase.tensor, ybase.offset + p4,
                            [[ypitch, P], [W * KPAIRS, OYS], [KPAIRS, W]],
                        )
                        nc.vector.affine_then_add(yap, B_ap, w3, kb, k0)

                # HBM [n, oyblk, (oy',ox,p)=1024]
                oap = bass.AP(
                    out_d, cl * OUT_CSTRIDE,
                    [[OUT_NSTRIDE, N], [OYS * W * KPAIRS, OYB], [1, OYS * W * KPAIRS]],
                )
                if merged:
                    pending.append((w_all, v_all, yc, oap, "gps" if mode == "gps_m" else "dve"))
                    if len(pending) > pipe_d:
                        flush_one()
                else:
                    nc.sync.dma_start(oap, ybase)
            while pending:
                flush_one()
    nc.finalize()  # Bacc: splits >1-wait syncs into event semaphores
    last_eng_ns[core] = dict(eng_ns)
    return nc


def _prep_inputs(x, weights, selection):
    x = np.ascontiguousarray(np.asarray(x, dtype=np.float32))
    weights = np.asarray(weights, dtype=np.float32)
    selection = np.asarray(selection, dtype=np.int32)

    # coefficients: softmax over 16 logic ops folded into {1,a,b,ab} basis
    w64 = weights.astype(np.float64)
    e = np.exp(w64 - w64.max(axis=-1, keepdims=True))
    prob = e / e.sum(axis=-1, keepdims=True)
    coef = (prob @ OP_COEFFS).astype(np.float32)  # [C_OUT, 4, 4]

    ch = ((selection >> 16) & 0xFFFF).astype(np.int64)
    ry = ((selection >> 8) & 0xFF).astype(np.int64)
    rx = (selection & 0xFF).astype(np.int64)

    # halo layout: xh[q=(n,oyblk), ch, r, w] = xpad[n, ch, oyblk*8+r, w]
    xpad = np.zeros((N, C_IN, H + 2, W + 2), dtype=np.float32)
    xpad[:, :, 1 : H + 1, 1 : W + 1] = x
    xh = np.empty((N, OYB, C_IN, HALO, W34), dtype=np.float16)
    for b in range(OYB):
        xh[:, b] = xpad[:, :, b * OYS : b * OYS + HALO, :]
    # per-core copies with channels permuted most-used-first so the load
    # cascade is a few big contiguous DMAs
    xh_cores = []
    for k in range(N_CORES):
        perm, _ = core_channel_perm(k, ch)
        xh_cores.append(np.ascontiguousarray(xh[:, :, perm].reshape(P, XFREE)))
    return xh_cores, ch, ry, rx, coef


def kernel(x, weights, selection):
    assert x.shape == (N, C_IN, H, W), x.shape
    assert weights.shape == (C_OUT, 4, 16), weights.shape
    assert selection.shape == (C_OUT, 8), selection.shape

    xh_cores, ch, ry, rx, coef = _prep_inputs(x, weights, selection)

    try:
        from concourse.timeline_sim import TimelineSim
    except Exception:  # noqa: BLE001
        TimelineSim = None
    # Per-core auto-tune: each core's selection pattern favors different
    # schedule knobs; build candidates and keep the TimelineSim-fastest.
    cands = CFG.get(
        "tune_candidates",
        (
            {},
            {"f_ata": 0.283, "uv_act": 0.395, "tail_ata": 1, "tp_bufs": 6,
             "wv_bufs": 8, "yc_bufs": 8},
            {"load_piece_ch": 6},
            {"f_ata": 0.26},
            {"f_ata": 0.31},
            {"uv_act": 0.40},
            {"uv_act": 0.50},
            {"uv_act": 0.395, "tail_ata": 1},
        ),
    )
    progs = []
    base_cfg = dict(CFG)
    for k in range(N_CORES):
        best = None
        for cfg_delta in cands if TimelineSim is not None else ({},):
            CFG.clear()
            CFG.update(base_cfg)
            CFG.update(cfg_delta)
            nc = build_core_program(k, ch, ry, rx, coef)
            ns = None
            if TimelineSim is not None:
                try:
                    ns = TimelineSim(nc, trace=False).simulate()
                except Exception:  # noqa: BLE001
                    ns = None
            if best is None or (ns is not None and best[0] is not None and ns < best[0]):
                best = (ns, nc)
            if ns is None:
                break
        progs.append(best[1])
        last_model_ns[k] = best[0]
    CFG.clear()
    CFG.update(base_cfg)

    import jax

    devices = jax.devices()
    assert len(devices) >= N_CORES, devices

    outs = [None] * N_CORES
    errs = [None] * N_CORES
    # NTFF tracing needs axon hooks that aren't present in this container —
    # make sure run_bass_kernel_spmd never tries (BASS_TRACE in env would).
    os.environ["BASS_NEVER_TRACE"] = "1"

    def run_one(k):
        try:
            with jax.default_device(devices[k]):
                res = bass_utils.run_bass_kernel_spmd(
                    progs[k], [{"xh": xh_cores[k]}], core_ids=[k]
                )
            last_results[k] = res
            outs[k] = res.results[0]["out"]
        except Exception as e:  # noqa: BLE001
            errs[k] = e

    threads = [threading.Thread(target=run_one, args=(k,)) for k in range(N_CORES)]
    for t in threads:
        t.start()
    for t in threads:
        t.join()
    for k, e in enumerate(errs):
        if e is not None:
            raise RuntimeError(f"core {k} failed") from e

    y = np.empty((N, C_OUT, H, W, KPAIRS), dtype=np.float32)
    for k in range(N_CORES):
        y[:, k * CPC : (k + 1) * CPC] = np.asarray(outs[k], dtype=np.float32)
    return y


# revision 58
# speedup vs baseline: 1.4483x; 1.0026x over previous
"""Trainium2 Bass kernel for nn_ConvLogicLayer.

Computes y[n,c,oy,ox,p] = k0 + ka*A + kb*B + kab*A*B where A/B are
shifted-window gathers of input channels (per the packed `selection`),
and k* are per-(c,p) coefficients derived from softmax(weights) @ OP_COEFFS.

Strategy (v2, fp16, multi-engine LP balance):
  - Shard C_out (512) across 8 cores -> 64 output channels per core.
  - Specialized per-core program: gather indices and coefficients baked
    into the instruction stream (static APs + immediate scalars).
  - SBUF layout: partition q = n*4 + oyblk (32 images x 4 row-blocks),
    free dim = 64 input channels x 10 halo rows x 34 padded cols, fp16,
    channels permuted per-core most-used-first.  A shifted 8x32 window
    for any (ch,ry,rx) is a static 3D AP.  The load streams as small
    contiguous piece-DMAs; channels are processed in completion-tier
    order so the in-order engine queues never wait on a late channel.
  - fp16 everywhere: DVE tensor_scalar runs in 4x mode (127ns/[128,256]),
    tensor_tensor in 2x mode (194ns), DMA bytes halve (in 5.6MB, out
    16.8MB per core).  Host converts the fp16 output to f32; rel err vs
    the f32 reference is ~1e-3 (tolerance 2e-2).
  - Per pair (c,p): u = kab*B + ka (DVE ts or ACT activation), then
    w = u*A (DVE tt).  y two ways, split by an offline LP over the
    per-op engine costs (DVE/ACT/GPSIMD finish together, ~98us loads):
      * f_ata of channels: per-pair y = (kb*B + k0) + w via the
        AFFINE_THEN_ADD custom DVE uop (strided p-interleaved write,
        v folded free; u on ACT for these channels);
      * the rest: v = kb*B + k0 (uv_act of u/v on ACT, rest DVE) into
        v_all, then ONE GPSIMD tensor_tensor [128,1024] per channel
        computes yc = w_all + v_all with the p-interleave expressed in
        the APs.  (scalar_tensor_tensor on Pool is f32-only in the real
        backend -- tensor_tensor ADD is the fp16-legal form.)
  - y written p-interleaved so the per-channel output DMA (256KB fp16)
    is 2KB-contiguous per partition in HBM and overlaps compute.
  - Per-core auto-tune over schedule knobs via TimelineSim.
"""

import contextlib
import os
import sys
import threading

import numpy as np

for _p in ("/opt/trn_rl_repo",):
    if _p not in sys.path and os.path.isdir(_p):
        sys.path.insert(0, _p)

import concourse.bass as bass
import concourse.bacc as bacc
import concourse.mybir as mybir
from concourse.tile import TileContext
from concourse import bass_utils

# Problem constants (hardcoded per spec)
N, C_IN, H, W = 32, 64, 32, 32
C_OUT, KPAIRS = 512, 4
N_CORES = 8
CPC = C_OUT // N_CORES  # channels per core

P = 128          # partitions = (n=32) x (oyblk=4)
OYB = 4          # oy blocks per image
OYS = 8          # oy rows per block
HALO = 10        # rows stored per block (8 + 2 halo)
W34 = 34         # padded width
CHSZ = HALO * W34           # 340 elems per (q, channel)
XFREE = C_IN * CHSZ         # 21760 elems per partition
OUT_CSTRIDE = H * W * KPAIRS          # 4096
OUT_NSTRIDE = CPC * OUT_CSTRIDE       # 262144
SPP = OYS * W    # 256 elems per (partition, pair)

OP_COEFFS = np.array([
    [0.0, 0.0, 0.0, 0.0], [0.0, 0.0, 0.0, 1.0], [0.0, 1.0, 0.0, -1.0],
    [0.0, 1.0, 0.0, 0.0], [0.0, 0.0, 1.0, -1.0], [0.0, 0.0, 1.0, 0.0],
    [0.0, 1.0, 1.0, -2.0], [0.0, 1.0, 1.0, -1.0], [1.0, -1.0, -1.0, 1.0],
    [1.0, -1.0, -1.0, 2.0], [1.0, 0.0, -1.0, 0.0], [1.0, 0.0, -1.0, 1.0],
    [1.0, -1.0, 0.0, 0.0], [1.0, -1.0, 0.0, 1.0], [1.0, 0.0, 0.0, -1.0],
    [1.0, 0.0, 0.0, 0.0],
], dtype=np.float64)

MULT = mybir.AluOpType.mult
ADD = mybir.AluOpType.add
COPY = mybir.ActivationFunctionType.Copy
F16 = mybir.dt.float16

# TimelineSim per-op costs (fp16, [128,256] unless noted)
DVE_TS = 127.0    # tensor_scalar, 4x mode
DVE_TT = 194.0    # tensor_tensor contiguous, 2x mode
DVE_ATA = 327.0   # AFFINE_THEN_ADD custom uop (any stride)
ACT_TS = 398.0    # activation copy w/ scale+bias
GPS_TS = 451.0    # gpsimd tensor_scalar
GPS_TT_M = 2127.0   # gpsimd tensor_tensor [128,1024] merged y (stt is f32-only)
GPS_TT_S = 603.0    # gpsimd tensor_tensor [128,256] per-pair y
DVE_TT_M = 1127.0   # DVE tensor_tensor [128,1024] merged y (1x, strided in)

CFG = {
    "load_piece_ch": 4,  # channels per streamed load DMA
    "load_cascade": None,
    "tp_bufs": 8,    # u/w scratch tiles
    "wv_bufs": 5,    # w_all/v_all channel tiles (GPS merged-y inputs)
    "yc_bufs": 12,   # output channel tiles
    "pipe_depth": 0,  # channels of consumer delay (0 = inline)
    "head_ata": 0,   # first channels forced all-DVE
    "tail_ata": 0,   # last channels forced all-DVE
    "head_merged": 0,
    "gps_v_head": 0,  # first channels with v forced to GPSIMD
    "gps_single": 0,  # per-pair GPS y instead of per-channel merged
    "gps_half": 0,    # split merged y into two half-channel ops
    "auto_quota": 0,
    "f_ata": 0.28,    # fraction of channels on the all-DVE ATA path
    "uv_act": 0.445,  # fraction of merged-channel u/v ops on ACT
}

last_results = [None] * N_CORES  # BassKernelResults per core (for profiling)
last_model_ns = [None] * N_CORES  # per-core TimelineSim estimate
last_eng_ns = [None] * N_CORES  # per-core greedy engine-load projection


def core_channel_perm(core, ch):
    """Input-channel permutation for `core`: most-used first.

    Returns (perm, inv) with perm[pos] = original channel stored at slot pos
    in this core's xh, inv[orig] = slot.
    """
    use_cnt = [0] * C_IN
    for cl in range(CPC):
        for j in range(2 * KPAIRS):
            use_cnt[int(ch[core * CPC + cl, j])] += 1
    perm = sorted(range(C_IN), key=lambda i: (-use_cnt[i], i))
    inv = [0] * C_IN
    for pos, orig in enumerate(perm):
        inv[orig] = pos
    return perm, inv


def build_core_program(core, ch, ry, rx, coef):
    """One specialized Bass program for `core` (channels core*CPC..+CPC).

    The host stores this core's xh with channels permuted most-used-first
    (core_channel_perm); `ch` is remapped here to slot indices so the
    cascade [s0, s1, ...] loads contiguous slabs front-to-back.
    """
    nc = bacc.Bacc("TRN2", target_bir_lowering=False)
    xh_d = nc.dram_tensor("xh", [P, XFREE], F16, kind="ExternalInput")
    out_d = nc.dram_tensor(
        "out", [N, CPC, H, W, KPAIRS], F16, kind="ExternalOutput"
    )

    eng_ns = {
        "dve": float(CFG.get("seed_dve", 0.0)),
        "act": float(CFG.get("seed_act", 0.0)),
        "gps": float(CFG.get("seed_gps", 0.0)),
    }

    with TileContext(nc) as tc:
        with (
            tc.tile_pool(name="xp", bufs=1) as xpool,
            tc.tile_pool(name="tp", bufs=CFG["tp_bufs"]) as tpool,
            tc.tile_pool(name="wv", bufs=CFG["wv_bufs"]) as wvpool,
            tc.tile_pool(name="yp", bufs=CFG["yc_bufs"]) as ypool,
        ):
            xh = xpool.tile([P, XFREE], F16)
            # ch remapped to this core's permuted slots (most-used first).
            _, inv = core_channel_perm(core, ch)
            chv = {
                (cl, j): inv[int(ch[core * CPC + cl, j])]
                for cl in range(CPC)
                for j in range(2 * KPAIRS)
            }
            # Streamed load: many small contiguous piece-DMAs front-to-back
            # (most-used slots first).  Pairs/channels are ordered by the
            # piece at which both operands are resident, so the in-order
            # engine queues never head-of-line block on a late channel.
            sizes = CFG.get("load_cascade")
            if not sizes:
                g = CFG.get("load_piece_ch", 4)
                sizes = [g] * (C_IN // g) + ([C_IN % g] if C_IN % g else [])
            bounds = []
            acc = 0
            for sz in sizes:
                acc += sz
                bounds.append(acc)
            assert bounds[-1] == C_IN, bounds

            def tier_of_slot(slot):
                for t, b in enumerate(bounds):
                    if slot < b:
                        return t
                raise AssertionError(slot)

            ptier = {
                (cl, p4): max(
                    tier_of_slot(chv[(cl, 2 * p4)]), tier_of_slot(chv[(cl, 2 * p4 + 1)])
                )
                for cl in range(CPC)
                for p4 in range(KPAIRS)
            }
            tiers = {cl: sorted(ptier[(cl, p4)] for p4 in range(KPAIRS)) for cl in range(CPC)}
            # sort by completion tier (max first): a channel is consumable
            # only once its LAST pair's inputs arrive, so straggler channels
            # must not sit at the front of the in-order queues.
            cl_order = sorted(range(CPC), key=lambda cl: tiers[cl][::-1])
            p4_order = {
                cl: sorted(range(KPAIRS), key=lambda p4: ptier[(cl, p4)])
                for cl in range(CPC)
            }
            lo = 0
            for b in bounds:
                nc.sync.dma_start(
                    xh[:, lo * CHSZ : b * CHSZ], xh_d[:, lo * CHSZ : b * CHSZ]
                )
                lo = b
            base = xh[:]
            pitch = base.ap[0][0]
            tens = base.tensor
            base_off = base.offset

            # Software pipelining: emit each channel's producers (u/w/v) now
            # but its consumers (merged gstt + output DMA) D channels later,
            # so no engine's queue head waits on a just-issued cross-engine
            # dependency.
            pipe_d = CFG.get("pipe_depth", 3)
            pending = []  # (w_all, v_all, yc, oap, y_eng) awaiting merge+DMA
            st = {"mp": 0, "ua": 0, "va": 0}  # merged-pair / ACT-quota counters

            def flush_one():
                w_all, v_all, yc_t, oap_t, y_eng, pos_t = pending.pop(0)
                yb = yc_t[:]
                yp_ = yb.ap[0][0]
                wb = w_all[:]
                vb = v_all[:]
                halves = CFG.get("gps_half", 0) or pos_t < CFG.get("half_head", 0)
                nh = 2 if halves else 1
                pk = KPAIRS // nh
                for h in range(nh):
                    w_ap = bass.AP(wb.tensor, wb.offset + h * pk * SPP,
                                   [[wb.ap[0][0], P], [1, SPP], [SPP, pk]])
                    v_ap = bass.AP(vb.tensor, vb.offset + h * pk * SPP,
                                   [[vb.ap[0][0], P], [1, SPP], [SPP, pk]])
                    y_ap = bass.AP(yb.tensor, yb.offset + h * pk,
                                   [[yp_, P], [KPAIRS, SPP], [1, pk]])
                    if y_eng == "gps":
                        nc.gpsimd.tensor_tensor(y_ap, w_ap, v_ap, ADD)
                    else:
                        nc.vector.tensor_tensor(y_ap, w_ap, v_ap, ADD)
                nc.sync.dma_start(oap_t, yb)

            for cl in cl_order:
                c = core * CPC + cl
                yc = ypool.tile([P, OYS * W * KPAIRS], F16, tag="yc")
                ybase = yc[:]
                ypitch = ybase.ap[0][0]

                # channel mode: merged y on GPS vs per-pair ATA on DVE.
                # Project both and keep whichever minimizes the makespan.
                # LP-quota assignment: mode basis is m3 (u ACT + all-DVE ATA),
                # m5 (u/v DVE + y GPS-merged), m7 (u/v ACT + y GPS-merged).
                # Solved so each engine's start offset + load is equal --
                # engines finish together rather than having equal totals.
                pos = cl_order.index(cl)
                f_ata = CFG.get("f_ata", 0.281)
                uv_act = CFG.get("uv_act", 0.473)
                if CFG.get("auto_quota", 0):
                    M = np.array([
                        [521.0, 448.0, 194.0, -1.0],
                        [398.0, 0.0, 796.0, -1.0],
                        [0.0, 532.0, 532.0, -1.0],
                        [1.0, 1.0, 1.0, 0.0],
                    ])
                    rhs = np.array([
                        -float(CFG.get("off_dve", 6500.0)),
                        -float(CFG.get("off_act", 6500.0)),
                        -float(CFG.get("off_gps", 11000.0)),
                        float(C_OUT // N_CORES * KPAIRS),
                    ])
                    x3, x5, x7, _T = np.linalg.solve(M, rhs)
                    x3, x5, x7 = max(x3, 0.0), max(x5, 0.0), max(x7, 0.0)
                    tot = x3 + x5 + x7
                    f_ata = x3 / tot
                    uv_act = x7 / max(x5 + x7, 1e-9)
                n_ata = int(round(f_ata * CPC))
                tail = min(CFG.get("tail_ata", 0), n_ata)
                head_m = CFG.get("head_merged", 0)
                spread = n_ata - tail
                span = CPC - tail - head_m
                if pos >= CPC - tail:
                    is_ata = True
                elif pos < head_m or span <= 0:
                    is_ata = False
                else:
                    q = pos - head_m
                    is_ata = int(q * spread / span) < int((q + 1) * spread / span)
                if pos < CFG.get("head_ata", 0):
                    is_ata = True
                if is_ata:
                    mode = "ata"
                elif pos < CFG.get("dvem_head", 0):
                    # head channels' merged y on DVE: GPS is still ramping in,
                    # so this trims GPS's total without delaying its start.
                    mode = "dve_m"
                elif CFG.get("gps_single", 0):
                    mode = "gps_s"  # per-pair gtt, strided out (no w_all/v_all)
                else:
                    mode = "gps_m"
                merged = mode in ("gps_m", "dve_m")
                uplace, vplace = [], []
                for _ in range(KPAIRS):
                    if is_ata:
                        uplace.append("act")
                        eng_ns["act"] += ACT_TS
                        eng_ns["dve"] += DVE_TT + DVE_ATA
                        continue
                    if mode == "gps_s":
                        eng_ns["gps"] += GPS_TT_S
                    st["mp"] += 1
                    # u placement against the ACT quota
                    if st["ua"] < uv_act * st["mp"]:
                        uplace.append("act")
                        st["ua"] += 1
                        eng_ns["act"] += ACT_TS
                    else:
                        uplace.append("dve")
                        eng_ns["dve"] += DVE_TS
                    eng_ns["dve"] += DVE_TT
                    if pos < CFG.get("gps_v_head", 0):
                        vplace.append("gps")
                        eng_ns["gps"] += GPS_TS
                    elif st["va"] < uv_act * st["mp"]:
                        vplace.append("act")
                        st["va"] += 1
                        eng_ns["act"] += ACT_TS
                    else:
                        vplace.append("dve")
                        eng_ns["dve"] += DVE_TS
                if merged:
                    if mode == "dve_m":
                        eng_ns["dve"] += DVE_TT_M
                    else:
                        eng_ns["gps"] += 2222.0 if CFG.get("gps_half", 0) else GPS_TT_M
                    w_all = wvpool.tile([P, KPAIRS * SPP], F16, tag="wa")
                    v_all = wvpool.tile([P, KPAIRS * SPP], F16, tag="va")

                # hoist the first merged channels' producer chains so GPSIMD's
                # first merged op fires as early as possible
                hstack = contextlib.ExitStack()
                if merged and pos < CFG.get("hoist_head", 0):
                    hstack.enter_context(tc.high_priority())
                for i, p4 in enumerate(p4_order[cl]):
                    ka_, kb_ = 2 * p4, 2 * p4 + 1
                    offA = base_off + chv[(cl, ka_)] * CHSZ + int(ry[c, ka_]) * W34 + int(rx[c, ka_])
                    offB = base_off + chv[(cl, kb_)] * CHSZ + int(ry[c, kb_]) * W34 + int(rx[c, kb_])
                    A_ap = bass.AP(tens, offA, [[pitch, P], [W34, OYS], [1, W]])
                    B_ap = bass.AP(tens, offB, [[pitch, P], [W34, OYS], [1, W]])

                    k0 = float(coef[c, p4, 0])
                    ka = float(coef[c, p4, 1])
                    kb = float(coef[c, p4, 2])
                    kab = float(coef[c, p4, 3])

                    u = tpool.tile([P, SPP], F16, tag="u")
                    u3 = u[:].rearrange("p (a b) -> p a b", b=W)
                    # u = kab*B + ka
                    ue = uplace[i]
                    if ue == "act":
                        nc.scalar.activation(u3, B_ap, COPY, bias=ka, scale=kab)
                    elif ue == "gps":
                        nc.gpsimd.tensor_scalar(u3, B_ap, kab, ka, MULT, ADD)
                    else:
                        nc.vector.tensor_scalar(u3, B_ap, kab, ka, MULT, ADD)

                    if merged:
                        wsl = w_all[:, p4 * SPP : (p4 + 1) * SPP]
                        w3 = wsl.rearrange("p (a b) -> p a b", b=W)
                        # w = u*A  (DVE tt, 2x, contiguous out)
                        nc.vector.tensor_tensor(w3, u3, A_ap, MULT)
                        vsl = v_all[:, p4 * SPP : (p4 + 1) * SPP]
                        v3 = vsl.rearrange("p (a b) -> p a b", b=W)
                        e = vplace[i]
                        if e == "act":
                            nc.scalar.activation(v3, B_ap, COPY, bias=k0, scale=kb)
                        elif e == "gps":
                            nc.gpsimd.tensor_scalar(v3, B_ap, kb, k0, MULT, ADD)
                        else:
                            nc.vector.tensor_scalar(v3, B_ap, kb, k0, MULT, ADD)
                    elif mode == "gps_s":
                        w = tpool.tile([P, SPP], F16, tag="w")
                        v = tpool.tile([P, SPP], F16, tag="v")
                        w3 = w[:].rearrange("p (a b) -> p a b", b=W)
                        v3 = v[:].rearrange("p (a b) -> p a b", b=W)
                        nc.vector.tensor_tensor(w3, u3, A_ap, MULT)
                        e = vplace[i]
                        if e == "act":
                            nc.scalar.activation(v3, B_ap, COPY, bias=k0, scale=kb)
                        elif e == "gps":
                            nc.gpsimd.tensor_scalar(v3, B_ap, kb, k0, MULT, ADD)
                        else:
                            nc.vector.tensor_scalar(v3, B_ap, kb, k0, MULT, ADD)
                        yap = bass.AP(
                            ybase.tensor, ybase.offset + p4,
                            [[ypitch, P], [W * KPAIRS, OYS], [KPAIRS, W]],
                        )
                        nc.gpsimd.tensor_tensor(yap, w3, v3, ADD)
                    else:
                        w = tpool.tile([P, SPP], F16, tag="w")
                        w3 = w[:].rearrange("p (a b) -> p a b", b=W)
                        nc.vector.tensor_tensor(w3, u3, A_ap, MULT)
                        # y = (kb*B + k0) + w, p-interleaved into yc (custom uop)
                        yap = bass.AP(
                            ybase.tensor, ybase.offset + p4,
                            [[ypitch, P], [W * KPAIRS, OYS], [KPAIRS, W]],
                        )
                        nc.vector.affine_then_add(yap, B_ap, w3, kb, k0)

                hstack.close()
                # HBM [n, oyblk, (oy',ox,p)=1024]
                oap = bass.AP(
                    out_d, cl * OUT_CSTRIDE,
                    [[OUT_NSTRIDE, N], [OYS * W * KPAIRS, OYB], [1, OYS * W * KPAIRS]],
                )
                if merged:
                    pending.append((w_all, v_all, yc, oap, "gps" if mode == "gps_m" else "dve", pos))
                    if len(pending) > pipe_d:
                        flush_one()
                else:
                    nc.sync.dma_start(oap, ybase)
            while pending:
                flush_one()
    nc.finalize()  # Bacc: splits >1-wait syncs into event semaphores
    last_eng_ns[core] = dict(eng_ns)
    return nc


def _prep_inputs(x, weights, selection):
    x = np.ascontiguousarray(np.asarray(x, dtype=np.float32))
    weights = np.asarray(weights, dtype=np.float32)
    selection = np.asarray(selection, dtype=np.int32)

    # coefficients: softmax over 16 logic ops folded into {1,a,b,ab} basis
    w64 = weights.astype(np.float64)
    e = np.exp(w64 - w64.max(axis=-1, keepdims=True))
    prob = e / e.sum(axis=-1, keepdims=True)
    coef = (prob @ OP_COEFFS).astype(np.float32)  # [C_OUT, 4, 4]

    ch = ((selection >> 16) & 0xFFFF).astype(np.int64)
    ry = ((selection >> 8) & 0xFF).astype(np.int64)
    rx = (selection & 0xFF).astype(np.int64)

    # halo layout: xh[q=(n,oyblk), ch, r, w] = xpad[n, ch, oyblk*8+r, w]
    xpad = np.zeros((N, C_IN, H + 2, W + 2), dtype=np.float32)
    xpad[:, :, 1 : H + 1, 1 : W + 1] = x
    xh = np.empty((N, OYB, C_IN, HALO, W34), dtype=np.float16)
    for b in range(OYB):
        xh[:, b] = xpad[:, :, b * OYS : b * OYS + HALO, :]
    # per-core copies with channels permuted most-used-first so the load
    # cascade is a few big contiguous DMAs
    xh_cores = []
    for k in range(N_CORES):
        perm, _ = core_channel_perm(k, ch)
        xh_cores.append(np.ascontiguousarray(xh[:, :, perm].reshape(P, XFREE)))
    return xh_cores, ch, ry, rx, coef


def kernel(x, weights, selection):
    assert x.shape == (N, C_IN, H, W), x.shape
    assert weights.shape == (C_OUT, 4, 16), weights.shape
    assert selection.shape == (C_OUT, 8), selection.shape

    xh_cores, ch, ry, rx, coef = _prep_inputs(x, weights, selection)

    try:
        from concourse.timeline_sim import TimelineSim
    except Exception:  # noqa: BLE001
        TimelineSim = None
    # Per-core auto-tune: each core's selection pattern favors different
    # schedule knobs; build candidates and keep the TimelineSim-fastest.
    cands = CFG.get(
        "tune_candidates",
        (
            {},
            {"half_head": 2},
            {"f_ata": 0.283, "uv_act": 0.415, "gps_v_head": 1, "tail_ata": 1,
             "tp_bufs": 6, "wv_bufs": 8, "yc_bufs": 9},
            {"f_ata": 0.283, "uv_act": 0.395, "tail_ata": 1, "tp_bufs": 6,
             "wv_bufs": 8, "yc_bufs": 8},
            {"f_ata": 0.283, "uv_act": 0.395, "tail_ata": 1, "tp_bufs": 6,
             "wv_bufs": 8, "yc_bufs": 8, "half_head": 2},
            {"load_piece_ch": 6},
            {"f_ata": 0.26, "half_head": 2},
            {"f_ata": 0.31},
            {"uv_act": 0.40, "half_head": 2},
            {"uv_act": 0.50},
            {"uv_act": 0.395, "tail_ata": 1, "half_head": 2},
        ),
    )
    progs = []
    base_cfg = dict(CFG)
    for k in range(N_CORES):
        best = None
        for cfg_delta in cands if TimelineSim is not None else ({},):
            CFG.clear()
            CFG.update(base_cfg)
            CFG.update(cfg_delta)
            nc = build_core_program(k, ch, ry, rx, coef)
            ns = None
            if TimelineSim is not None:
                try:
                    ns = TimelineSim(nc, trace=False).simulate()
                except Exception:  # noqa: BLE001
                    ns = None
            if best is None or (ns is not None and best[0] is not None and ns < best[0]):
                best = (ns, nc)
            if ns is None:
                break
        progs.append(best[1])
        last_model_ns[k] = best[0]
    CFG.clear()
    CFG.update(base_cfg)

    import jax

    devices = jax.devices()
    assert len(devices) >= N_CORES, devices

    outs = [None] * N_CORES
    errs = [None] * N_CORES
    # NTFF tracing needs axon hooks that aren't present in this container —
    # make sure run_bass_kernel_spmd never tries (BASS_TRACE in env would).
    os.environ["BASS_NEVER_TRACE"] = "1"

    def run_one(k):
        try:
            with jax.default_device(devices[k]):
                res = bass_utils.run_bass_kernel_spmd(
                    progs[k], [{"xh": xh_cores[k]}], core_ids=[k]
                )
            last_results[k] = res
            outs[k] = res.results[0]["out"]
        except Exception as e:  # noqa: BLE001
            errs[k] = e

    threads = [threading.Thread(target=run_one, args=(k,)) for k in range(N_CORES)]
    for t in threads:
        t.start()
    for t in threads:
        t.join()
    for k, e in enumerate(errs):
        if e is not None:
            raise RuntimeError(f"core {k} failed") from e

    y = np.empty((N, C_OUT, H, W, KPAIRS), dtype=np.float32)
    for k in range(N_CORES):
        y[:, k * CPC : (k + 1) * CPC] = np.asarray(outs[k], dtype=np.float32)
    return y


# revision 61
# speedup vs baseline: 1.4499x; 1.0011x over previous
"""Trainium2 Bass kernel for nn_ConvLogicLayer.

Computes y[n,c,oy,ox,p] = k0 + ka*A + kb*B + kab*A*B where A/B are
shifted-window gathers of input channels (per the packed `selection`),
and k* are per-(c,p) coefficients derived from softmax(weights) @ OP_COEFFS.

Strategy (v2, fp16, multi-engine LP balance):
  - Shard C_out (512) across 8 cores -> 64 output channels per core.
  - Specialized per-core program: gather indices and coefficients baked
    into the instruction stream (static APs + immediate scalars).
  - SBUF layout: partition q = n*4 + oyblk (32 images x 4 row-blocks),
    free dim = 64 input channels x 10 halo rows x 34 padded cols, fp16,
    channels permuted per-core most-used-first.  A shifted 8x32 window
    for any (ch,ry,rx) is a static 3D AP.  The load streams as small
    contiguous piece-DMAs; channels are processed in completion-tier
    order so the in-order engine queues never wait on a late channel.
  - fp16 everywhere: DVE tensor_scalar runs in 4x mode (127ns/[128,256]),
    tensor_tensor in 2x mode (194ns), DMA bytes halve (in 5.6MB, out
    16.8MB per core).  Host converts the fp16 output to f32; rel err vs
    the f32 reference is ~1e-3 (tolerance 2e-2).
  - Per pair (c,p): u = kab*B + ka (DVE ts or ACT activation), then
    w = u*A (DVE tt).  y two ways, split by an offline LP over the
    per-op engine costs (DVE/ACT/GPSIMD finish together, ~98us loads):
      * f_ata of channels: per-pair y = (kb*B + k0) + w via the
        AFFINE_THEN_ADD custom DVE uop (strided p-interleaved write,
        v folded free; u on ACT for these channels);
      * the rest: v = kb*B + k0 (uv_act of u/v on ACT, rest DVE) into
        v_all, then ONE GPSIMD tensor_tensor [128,1024] per channel
        computes yc = w_all + v_all with the p-interleave expressed in
        the APs.  (scalar_tensor_tensor on Pool is f32-only in the real
        backend -- tensor_tensor ADD is the fp16-legal form.)
  - y written p-interleaved so the per-channel output DMA (256KB fp16)
    is 2KB-contiguous per partition in HBM and overlaps compute.
  - Per-core auto-tune over schedule knobs via TimelineSim.
"""

import contextlib
import os
import sys
import threading

import numpy as np

for _p in ("/opt/trn_rl_repo",):
    if _p not in sys.path and os.path.isdir(_p):
        sys.path.insert(0, _p)

import concourse.bass as bass
import concourse.bacc as bacc
import concourse.mybir as mybir
from concourse.tile import TileContext
from concourse import bass_utils

# Problem constants (hardcoded per spec)
N, C_IN, H, W = 32, 64, 32, 32
C_OUT, KPAIRS = 512, 4
N_CORES = 8
CPC = C_OUT // N_CORES  # channels per core

P = 128          # partitions = (n=32) x (oyblk=4)
OYB = 4          # oy blocks per image
OYS = 8          # oy rows per block
HALO = 10        # rows stored per block (8 + 2 halo)
W34 = 34         # padded width
CHSZ = HALO * W34           # 340 elems per (q, channel)
XFREE = C_IN * CHSZ         # 21760 elems per partition
OUT_CSTRIDE = H * W * KPAIRS          # 4096
OUT_NSTRIDE = CPC * OUT_CSTRIDE       # 262144
SPP = OYS * W    # 256 elems per (partition, pair)

OP_COEFFS = np.array([
    [0.0, 0.0, 0.0, 0.0], [0.0, 0.0, 0.0, 1.0], [0.0, 1.0, 0.0, -1.0],
    [0.0, 1.0, 0.0, 0.0], [0.0, 0.0, 1.0, -1.0], [0.0, 0.0, 1.0, 0.0],
    [0.0, 1.0, 1.0, -2.0], [0.0, 1.0, 1.0, -1.0], [1.0, -1.0, -1.0, 1.0],
    [1.0, -1.0, -1.0, 2.0], [1.0, 0.0, -1.0, 0.0], [1.0, 0.0, -1.0, 1.0],
    [1.0, -1.0, 0.0, 0.0], [1.0, -1.0, 0.0, 1.0], [1.0, 0.0, 0.0, -1.0],
    [1.0, 0.0, 0.0, 0.0],
], dtype=np.float64)

MULT = mybir.AluOpType.mult
ADD = mybir.AluOpType.add
COPY = mybir.ActivationFunctionType.Copy
F16 = mybir.dt.float16

# TimelineSim per-op costs (fp16, [128,256] unless noted)
DVE_TS = 127.0    # tensor_scalar, 4x mode
DVE_TT = 194.0    # tensor_tensor contiguous, 2x mode
DVE_ATA = 327.0   # AFFINE_THEN_ADD custom uop (any stride)
ACT_TS = 398.0    # activation copy w/ scale+bias
GPS_TS = 451.0    # gpsimd tensor_scalar
GPS_TT_M = 2127.0   # gpsimd tensor_tensor [128,1024] merged y (stt is f32-only)
GPS_TT_S = 603.0    # gpsimd tensor_tensor [128,256] per-pair y
DVE_TT_M = 1127.0   # DVE tensor_tensor [128,1024] merged y (1x, strided in)

CFG = {
    "load_piece_ch": 4,  # channels per streamed load DMA
    "load_cascade": None,
    "tp_bufs": 8,    # u/w scratch tiles
    "wv_bufs": 5,    # w_all/v_all channel tiles (GPS merged-y inputs)
    "yc_bufs": 12,   # output channel tiles
    "pipe_depth": 0,  # channels of consumer delay (0 = inline)
    "head_ata": 0,   # first channels forced all-DVE
    "tail_ata": 0,   # last channels forced all-DVE
    "head_merged": 0,
    "gps_v_head": 0,  # first channels with v forced to GPSIMD
    "gps_single": 0,  # per-pair GPS y instead of per-channel merged
    "gps_half": 0,    # split merged y into two half-channel ops
    "auto_quota": 0,
    "f_ata": 0.28,    # fraction of channels on the all-DVE ATA path
    "uv_act": 0.445,  # fraction of merged-channel u/v ops on ACT
}

last_results = [None] * N_CORES  # BassKernelResults per core (for profiling)
last_model_ns = [None] * N_CORES  # per-core TimelineSim estimate
last_eng_ns = [None] * N_CORES  # per-core greedy engine-load projection


def core_channel_perm(core, ch):
    """Input-channel permutation for `core`: most-used first.

    With CFG['seed_channel'], the output channel needing the fewest distinct
    input channels gets its inputs at the very front, so its producer chain
    (and GPSIMD's first merged op) unlocks after the first 1-2 load pieces.

    Returns (perm, inv) with perm[pos] = original channel stored at slot pos
    in this core's xh, inv[orig] = slot.
    """
    use_cnt = [0] * C_IN
    need = []
    for cl in range(CPC):
        s = {int(ch[core * CPC + cl, j]) for j in range(2 * KPAIRS)}
        need.append(s)
        for j in range(2 * KPAIRS):
            use_cnt[int(ch[core * CPC + cl, j])] += 1
    head = []
    sc = CFG.get("seed_channel", 0)
    if isinstance(sc, (list, tuple)):
        sc = sc[core]
    if sc:
        best = min(range(CPC), key=lambda cl: len(need[cl]))
        head = sorted(need[best], key=lambda i: (-use_cnt[i], i))
    rest = [i for i in sorted(range(C_IN), key=lambda i: (-use_cnt[i], i)) if i not in set(head)]
    perm = head + rest
    inv = [0] * C_IN
    for pos, orig in enumerate(perm):
        inv[orig] = pos
    return perm, inv


def build_core_program(core, ch, ry, rx, coef):
    """One specialized Bass program for `core` (channels core*CPC..+CPC).

    The host stores this core's xh with channels permuted most-used-first
    (core_channel_perm); `ch` is remapped here to slot indices so the
    cascade [s0, s1, ...] loads contiguous slabs front-to-back.
    """
    nc = bacc.Bacc("TRN2", target_bir_lowering=False)
    xh_d = nc.dram_tensor("xh", [P, XFREE], F16, kind="ExternalInput")
    out_d = nc.dram_tensor(
        "out", [N, CPC, H, W, KPAIRS], F16, kind="ExternalOutput"
    )

    eng_ns = {
        "dve": float(CFG.get("seed_dve", 0.0)),
        "act": float(CFG.get("seed_act", 0.0)),
        "gps": float(CFG.get("seed_gps", 0.0)),
    }

    with TileContext(nc) as tc:
        with (
            tc.tile_pool(name="xp", bufs=1) as xpool,
            tc.tile_pool(name="tp", bufs=CFG["tp_bufs"]) as tpool,
            tc.tile_pool(name="wv", bufs=CFG["wv_bufs"]) as wvpool,
            tc.tile_pool(name="yp", bufs=CFG["yc_bufs"]) as ypool,
        ):
            xh = xpool.tile([P, XFREE], F16)
            # ch remapped to this core's permuted slots (most-used first).
            _, inv = core_channel_perm(core, ch)
            chv = {
                (cl, j): inv[int(ch[core * CPC + cl, j])]
                for cl in range(CPC)
                for j in range(2 * KPAIRS)
            }
            # Streamed load: many small contiguous piece-DMAs front-to-back
            # (most-used slots first).  Pairs/channels are ordered by the
            # piece at which both operands are resident, so the in-order
            # engine queues never head-of-line block on a late channel.
            sizes = CFG.get("load_cascade")
            if not sizes:
                g = CFG.get("load_piece_ch", 4)
                sizes = [g] * (C_IN // g) + ([C_IN % g] if C_IN % g else [])
            bounds = []
            acc = 0
            for sz in sizes:
                acc += sz
                bounds.append(acc)
            assert bounds[-1] == C_IN, bounds

            def tier_of_slot(slot):
                for t, b in enumerate(bounds):
                    if slot < b:
                        return t
                raise AssertionError(slot)

            ptier = {
                (cl, p4): max(
                    tier_of_slot(chv[(cl, 2 * p4)]), tier_of_slot(chv[(cl, 2 * p4 + 1)])
                )
                for cl in range(CPC)
                for p4 in range(KPAIRS)
            }
            tiers = {cl: sorted(ptier[(cl, p4)] for p4 in range(KPAIRS)) for cl in range(CPC)}
            # sort by completion tier (max first): a channel is consumable
            # only once its LAST pair's inputs arrive, so straggler channels
            # must not sit at the front of the in-order queues.
            cl_order = sorted(range(CPC), key=lambda cl: tiers[cl][::-1])
            p4_order = {
                cl: sorted(range(KPAIRS), key=lambda p4: ptier[(cl, p4)])
                for cl in range(CPC)
            }
            lo = 0
            for b in bounds:
                nc.sync.dma_start(
                    xh[:, lo * CHSZ : b * CHSZ], xh_d[:, lo * CHSZ : b * CHSZ]
                )
                lo = b
            base = xh[:]
            pitch = base.ap[0][0]
            tens = base.tensor
            base_off = base.offset

            # Software pipelining: emit each channel's producers (u/w/v) now
            # but its consumers (merged gstt + output DMA) D channels later,
            # so no engine's queue head waits on a just-issued cross-engine
            # dependency.
            pipe_d = CFG.get("pipe_depth", 3)
            pending = []  # (w_all, v_all, yc, oap, y_eng) awaiting merge+DMA
            st = {"mp": 0, "ua": 0, "va": 0}  # merged-pair / ACT-quota counters

            def flush_one():
                w_all, v_all, yc_t, oap_t, y_eng, pos_t = pending.pop(0)
                yb = yc_t[:]
                yp_ = yb.ap[0][0]
                wb = w_all[:]
                vb = v_all[:]
                halves = CFG.get("gps_half", 0) or pos_t < CFG.get("half_head", 0)
                nh = 2 if halves else 1
                pk = KPAIRS // nh
                for h in range(nh):
                    w_ap = bass.AP(wb.tensor, wb.offset + h * pk * SPP,
                                   [[wb.ap[0][0], P], [1, SPP], [SPP, pk]])
                    v_ap = bass.AP(vb.tensor, vb.offset + h * pk * SPP,
                                   [[vb.ap[0][0], P], [1, SPP], [SPP, pk]])
                    y_ap = bass.AP(yb.tensor, yb.offset + h * pk,
                                   [[yp_, P], [KPAIRS, SPP], [1, pk]])
                    if y_eng == "gps":
                        nc.gpsimd.tensor_tensor(y_ap, w_ap, v_ap, ADD)
                    else:
                        nc.vector.tensor_tensor(y_ap, w_ap, v_ap, ADD)
                nc.sync.dma_start(oap_t, yb)

            for cl in cl_order:
                c = core * CPC + cl
                yc = ypool.tile([P, OYS * W * KPAIRS], F16, tag="yc")
                ybase = yc[:]
                ypitch = ybase.ap[0][0]

                # channel mode: merged y on GPS vs per-pair ATA on DVE.
                # Project both and keep whichever minimizes the makespan.
                # LP-quota assignment: mode basis is m3 (u ACT + all-DVE ATA),
                # m5 (u/v DVE + y GPS-merged), m7 (u/v ACT + y GPS-merged).
                # Solved so each engine's start offset + load is equal --
                # engines finish together rather than having equal totals.
                pos = cl_order.index(cl)
                f_ata = CFG.get("f_ata", 0.281)
                uv_act = CFG.get("uv_act", 0.473)
                if CFG.get("auto_quota", 0):
                    M = np.array([
                        [521.0, 448.0, 194.0, -1.0],
                        [398.0, 0.0, 796.0, -1.0],
                        [0.0, 532.0, 532.0, -1.0],
                        [1.0, 1.0, 1.0, 0.0],
                    ])
                    rhs = np.array([
                        -float(CFG.get("off_dve", 6500.0)),
                        -float(CFG.get("off_act", 6500.0)),
                        -float(CFG.get("off_gps", 11000.0)),
                        float(C_OUT // N_CORES * KPAIRS),
                    ])
                    x3, x5, x7, _T = np.linalg.solve(M, rhs)
                    x3, x5, x7 = max(x3, 0.0), max(x5, 0.0), max(x7, 0.0)
                    tot = x3 + x5 + x7
                    f_ata = x3 / tot
                    uv_act = x7 / max(x5 + x7, 1e-9)
                n_ata = int(round(f_ata * CPC))
                tail = min(CFG.get("tail_ata", 0), n_ata)
                head_m = CFG.get("head_merged", 0)
                spread = n_ata - tail
                span = CPC - tail - head_m
                if pos >= CPC - tail:
                    is_ata = True
                elif pos < head_m or span <= 0:
                    is_ata = False
                else:
                    q = pos - head_m
                    is_ata = int(q * spread / span) < int((q + 1) * spread / span)
                if pos < CFG.get("head_ata", 0):
                    is_ata = True
                if is_ata:
                    mode = "ata"
                elif pos < CFG.get("dvem_head", 0):
                    # head channels' merged y on DVE: GPS is still ramping in,
                    # so this trims GPS's total without delaying its start.
                    mode = "dve_m"
                elif CFG.get("gps_single", 0):
                    mode = "gps_s"  # per-pair gtt, strided out (no w_all/v_all)
                else:
                    mode = "gps_m"
                merged = mode in ("gps_m", "dve_m")
                uplace, vplace = [], []
                for _ in range(KPAIRS):
                    if is_ata:
                        uplace.append("act")
                        eng_ns["act"] += ACT_TS
                        eng_ns["dve"] += DVE_TT + DVE_ATA
                        continue
                    if mode == "gps_s":
                        eng_ns["gps"] += GPS_TT_S
                    st["mp"] += 1
                    # u placement against the ACT quota
                    if st["ua"] < uv_act * st["mp"]:
                        uplace.append("act")
                        st["ua"] += 1
                        eng_ns["act"] += ACT_TS
                    else:
                        uplace.append("dve")
                        eng_ns["dve"] += DVE_TS
                    eng_ns["dve"] += DVE_TT
                    if pos < CFG.get("gps_v_head", 0):
                        vplace.append("gps")
                        eng_ns["gps"] += GPS_TS
                    elif st["va"] < uv_act * st["mp"]:
                        vplace.append("act")
                        st["va"] += 1
                        eng_ns["act"] += ACT_TS
                    else:
                        vplace.append("dve")
                        eng_ns["dve"] += DVE_TS
                if merged:
                    if mode == "dve_m":
                        eng_ns["dve"] += DVE_TT_M
                    else:
                        eng_ns["gps"] += 2222.0 if CFG.get("gps_half", 0) else GPS_TT_M
                    w_all = wvpool.tile([P, KPAIRS * SPP], F16, tag="wa")
                    v_all = wvpool.tile([P, KPAIRS * SPP], F16, tag="va")

                # hoist the first merged channels' producer chains so GPSIMD's
                # first merged op fires as early as possible
                hstack = contextlib.ExitStack()
                if merged and pos < CFG.get("hoist_head", 0):
                    hstack.enter_context(tc.high_priority())
                for i, p4 in enumerate(p4_order[cl]):
                    ka_, kb_ = 2 * p4, 2 * p4 + 1
                    offA = base_off + chv[(cl, ka_)] * CHSZ + int(ry[c, ka_]) * W34 + int(rx[c, ka_])
                    offB = base_off + chv[(cl, kb_)] * CHSZ + int(ry[c, kb_]) * W34 + int(rx[c, kb_])
                    A_ap = bass.AP(tens, offA, [[pitch, P], [W34, OYS], [1, W]])
                    B_ap = bass.AP(tens, offB, [[pitch, P], [W34, OYS], [1, W]])

                    k0 = float(coef[c, p4, 0])
                    ka = float(coef[c, p4, 1])
                    kb = float(coef[c, p4, 2])
                    kab = float(coef[c, p4, 3])

                    u = tpool.tile([P, SPP], F16, tag="u")
                    u3 = u[:].rearrange("p (a b) -> p a b", b=W)
                    # u = kab*B + ka
                    ue = uplace[i]
                    if ue == "act":
                        nc.scalar.activation(u3, B_ap, COPY, bias=ka, scale=kab)
                    elif ue == "gps":
                        nc.gpsimd.tensor_scalar(u3, B_ap, kab, ka, MULT, ADD)
                    else:
                        nc.vector.tensor_scalar(u3, B_ap, kab, ka, MULT, ADD)

                    if merged:
                        wsl = w_all[:, p4 * SPP : (p4 + 1) * SPP]
                        w3 = wsl.rearrange("p (a b) -> p a b", b=W)
                        # w = u*A  (DVE tt, 2x, contiguous out)
                        nc.vector.tensor_tensor(w3, u3, A_ap, MULT)
                        vsl = v_all[:, p4 * SPP : (p4 + 1) * SPP]
                        v3 = vsl.rearrange("p (a b) -> p a b", b=W)
                        e = vplace[i]
                        if e == "act":
                            nc.scalar.activation(v3, B_ap, COPY, bias=k0, scale=kb)
                        elif e == "gps":
                            nc.gpsimd.tensor_scalar(v3, B_ap, kb, k0, MULT, ADD)
                        else:
                            nc.vector.tensor_scalar(v3, B_ap, kb, k0, MULT, ADD)
                    elif mode == "gps_s":
                        w = tpool.tile([P, SPP], F16, tag="w")
                        v = tpool.tile([P, SPP], F16, tag="v")
                        w3 = w[:].rearrange("p (a b) -> p a b", b=W)
                        v3 = v[:].rearrange("p (a b) -> p a b", b=W)
                        nc.vector.tensor_tensor(w3, u3, A_ap, MULT)
                        e = vplace[i]
                        if e == "act":
                            nc.scalar.activation(v3, B_ap, COPY, bias=k0, scale=kb)
                        elif e == "gps":
                            nc.gpsimd.tensor_scalar(v3, B_ap, kb, k0, MULT, ADD)
                        else:
                            nc.vector.tensor_scalar(v3, B_ap, kb, k0, MULT, ADD)
                        yap = bass.AP(
                            ybase.tensor, ybase.offset + p4,
                            [[ypitch, P], [W * KPAIRS, OYS], [KPAIRS, W]],
                        )
                        nc.gpsimd.tensor_tensor(yap, w3, v3, ADD)
                    else:
                        w = tpool.tile([P, SPP], F16, tag="w")
                        w3 = w[:].rearrange("p (a b) -> p a b", b=W)
                        nc.vector.tensor_tensor(w3, u3, A_ap, MULT)
                        # y = (kb*B + k0) + w, p-interleaved into yc (custom uop)
                        yap = bass.AP(
                            ybase.tensor, ybase.offset + p4,
                            [[ypitch, P], [W * KPAIRS, OYS], [KPAIRS, W]],
                        )
                        nc.vector.affine_then_add(yap, B_ap, w3, kb, k0)

                hstack.close()
                # HBM [n, oyblk, (oy',ox,p)=1024]
                oap = bass.AP(
                    out_d, cl * OUT_CSTRIDE,
                    [[OUT_NSTRIDE, N], [OYS * W * KPAIRS, OYB], [1, OYS * W * KPAIRS]],
                )
                if merged:
                    pending.append((w_all, v_all, yc, oap, "gps" if mode == "gps_m" else "dve", pos))
                    if len(pending) > pipe_d:
                        flush_one()
                else:
                    nc.sync.dma_start(oap, ybase)
            while pending:
                flush_one()
    nc.finalize()  # Bacc: splits >1-wait syncs into event semaphores
    last_eng_ns[core] = dict(eng_ns)
    return nc


def _prep_inputs(x, weights, selection):
    x = np.ascontiguousarray(np.asarray(x, dtype=np.float32))
    weights = np.asarray(weights, dtype=np.float32)
    selection = np.asarray(selection, dtype=np.int32)

    # coefficients: softmax over 16 logic ops folded into {1,a,b,ab} basis
    w64 = weights.astype(np.float64)
    e = np.exp(w64 - w64.max(axis=-1, keepdims=True))
    prob = e / e.sum(axis=-1, keepdims=True)
    coef = (prob @ OP_COEFFS).astype(np.float32)  # [C_OUT, 4, 4]

    ch = ((selection >> 16) & 0xFFFF).astype(np.int64)
    ry = ((selection >> 8) & 0xFF).astype(np.int64)
    rx = (selection & 0xFF).astype(np.int64)

    # halo layout: xh[q=(n,oyblk), ch, r, w] = xpad[n, ch, oyblk*8+r, w]
    xpad = np.zeros((N, C_IN, H + 2, W + 2), dtype=np.float32)
    xpad[:, :, 1 : H + 1, 1 : W + 1] = x
    xh = np.empty((N, OYB, C_IN, HALO, W34), dtype=np.float16)
    for b in range(OYB):
        xh[:, b] = xpad[:, :, b * OYS : b * OYS + HALO, :]
    # per-core copies with channels permuted most-used-first so the load
    # cascade is a few big contiguous DMAs
    xh_cores = []
    for k in range(N_CORES):
        perm, _ = core_channel_perm(k, ch)
        xh_cores.append(np.ascontiguousarray(xh[:, :, perm].reshape(P, XFREE)))
    return xh_cores, ch, ry, rx, coef


def kernel(x, weights, selection):
    assert x.shape == (N, C_IN, H, W), x.shape
    assert weights.shape == (C_OUT, 4, 16), weights.shape
    assert selection.shape == (C_OUT, 8), selection.shape

    try:
        from concourse.timeline_sim import TimelineSim
    except Exception:  # noqa: BLE001
        TimelineSim = None

    # Pre-pass: decide the per-core seed-channel layout flag (it changes the
    # host xh layout, so it must be fixed before _prep_inputs and never
    # toggled by the per-core schedule candidates below).
    if TimelineSim is not None and not isinstance(CFG.get("seed_channel"), (list, tuple)):
        _, ch_t, ry_t, rx_t, coef_t = _prep_inputs(x, weights, selection)
        flags = []
        base_cfg0 = dict(CFG)
        for k in range(N_CORES):
            scores = {}
            for flag in (0, 1):
                best = None
                for delta in ({}, {"half_head": 2}):
                    CFG.clear()
                    CFG.update(base_cfg0)
                    CFG.update(delta)
                    CFG["seed_channel"] = flag
                    try:
                        ns = TimelineSim(
                            build_core_program(k, ch_t, ry_t, rx_t, coef_t),
                            trace=False,
                        ).simulate()
                    except Exception:  # noqa: BLE001
                        ns = float("inf")
                    best = ns if best is None else min(best, ns)
                scores[flag] = best
            flags.append(1 if scores[1] < scores[0] else 0)
        CFG.clear()
        CFG.update(base_cfg0)
        CFG["seed_channel"] = tuple(flags)

    xh_cores, ch, ry, rx, coef = _prep_inputs(x, weights, selection)
    # Per-core auto-tune: each core's selection pattern favors different
    # schedule knobs; build candidates and keep the TimelineSim-fastest.
    cands = CFG.get(
        "tune_candidates",
        (
            {},
            {"half_head": 2},
            {"f_ata": 0.283, "uv_act": 0.415, "gps_v_head": 1, "tail_ata": 1,
             "tp_bufs": 6, "wv_bufs": 8, "yc_bufs": 9},
            {"f_ata": 0.283, "uv_act": 0.395, "tail_ata": 1, "tp_bufs": 6,
             "wv_bufs": 8, "yc_bufs": 8},
            {"f_ata": 0.283, "uv_act": 0.395, "tail_ata": 1, "tp_bufs": 6,
             "wv_bufs": 8, "yc_bufs": 8, "half_head": 2},
            {"load_piece_ch": 6},
            {"f_ata": 0.26, "half_head": 2},
            {"f_ata": 0.31},
            {"uv_act": 0.40, "half_head": 2},
            {"uv_act": 0.50},
            {"uv_act": 0.395, "tail_ata": 1, "half_head": 2},
        ),
    )
    progs = []
    base_cfg = dict(CFG)
    for k in range(N_CORES):
        best = None
        for cfg_delta in cands if TimelineSim is not None else ({},):
            CFG.clear()
            CFG.update(base_cfg)
            CFG.update(cfg_delta)
            nc = build_core_program(k, ch, ry, rx, coef)
            ns = None
            if TimelineSim is not None:
                try:
                    ns = TimelineSim(nc, trace=False).simulate()
                except Exception:  # noqa: BLE001
                    ns = None
            if best is None or (ns is not None and best[0] is not None and ns < best[0]):
                best = (ns, nc)
            if ns is None:
                break
        progs.append(best[1])
        last_model_ns[k] = best[0]
    CFG.clear()
    CFG.update(base_cfg)

    import jax

    devices = jax.devices()
    assert len(devices) >= N_CORES, devices

    outs = [None] * N_CORES
    errs = [None] * N_CORES
    # NTFF tracing needs axon hooks that aren't present in this container —
    # make sure run_bass_kernel_spmd never tries (BASS_TRACE in env would).
    os.environ["BASS_NEVER_TRACE"] = "1"

    def run_one(k):
        try:
            with jax.default_device(devices[k]):
                res = bass_utils.run_bass_kernel_spmd(
                    progs[k], [{"xh": xh_cores[k]}], core_ids=[k]
                )
            last_results[k] = res
            outs[k] = res.results[0]["out"]
        except Exception as e:  # noqa: BLE001
            errs[k] = e

    threads = [threading.Thread(target=run_one, args=(k,)) for k in range(N_CORES)]
    for t in threads:
        t.start()
    for t in threads:
        t.join()
    for k, e in enumerate(errs):
        if e is not None:
            raise RuntimeError(f"core {k} failed") from e

    y = np.empty((N, C_OUT, H, W, KPAIRS), dtype=np.float32)
    for k in range(N_CORES):
        y[:, k * CPC : (k + 1) * CPC] = np.asarray(outs[k], dtype=np.float32)
    return y


# revision 62
# speedup vs baseline: 1.4565x; 1.0046x over previous
"""Trainium2 Bass kernel for nn_ConvLogicLayer.

Computes y[n,c,oy,ox,p] = k0 + ka*A + kb*B + kab*A*B where A/B are
shifted-window gathers of input channels (per the packed `selection`),
and k* are per-(c,p) coefficients derived from softmax(weights) @ OP_COEFFS.

Strategy (v2, fp16, multi-engine LP balance):
  - Shard C_out (512) across 8 cores -> 64 output channels per core.
  - Specialized per-core program: gather indices and coefficients baked
    into the instruction stream (static APs + immediate scalars).
  - SBUF layout: partition q = n*4 + oyblk (32 images x 4 row-blocks),
    free dim = 64 input channels x 10 halo rows x 34 padded cols, fp16,
    channels permuted per-core most-used-first.  A shifted 8x32 window
    for any (ch,ry,rx) is a static 3D AP.  The load streams as small
    contiguous piece-DMAs; channels are processed in completion-tier
    order so the in-order engine queues never wait on a late channel.
  - fp16 everywhere: DVE tensor_scalar runs in 4x mode (127ns/[128,256]),
    tensor_tensor in 2x mode (194ns), DMA bytes halve (in 5.6MB, out
    16.8MB per core).  Host converts the fp16 output to f32; rel err vs
    the f32 reference is ~1e-3 (tolerance 2e-2).
  - Per pair (c,p): u = kab*B + ka (DVE ts or ACT activation), then
    w = u*A (DVE tt).  y two ways, split by an offline LP over the
    per-op engine costs (DVE/ACT/GPSIMD finish together, ~98us loads):
      * f_ata of channels: per-pair y = (kb*B + k0) + w via the
        AFFINE_THEN_ADD custom DVE uop (strided p-interleaved write,
        v folded free; u on ACT for these channels);
      * the rest: v = kb*B + k0 (uv_act of u/v on ACT, rest DVE) into
        v_all, then ONE GPSIMD tensor_tensor [128,1024] per channel
        computes yc = w_all + v_all with the p-interleave expressed in
        the APs.  (scalar_tensor_tensor on Pool is f32-only in the real
        backend -- tensor_tensor ADD is the fp16-legal form.)
  - y written p-interleaved so the per-channel output DMA (256KB fp16)
    is 2KB-contiguous per partition in HBM and overlaps compute.
  - Per-core auto-tune over schedule knobs via TimelineSim.
"""

import contextlib
import os
import sys
import threading

import numpy as np

for _p in ("/opt/trn_rl_repo",):
    if _p not in sys.path and os.path.isdir(_p):
        sys.path.insert(0, _p)

import concourse.bass as bass
import concourse.bacc as bacc
import concourse.mybir as mybir
from concourse.tile import TileContext
from concourse import bass_utils

# Problem constants (hardcoded per spec)
N, C_IN, H, W = 32, 64, 32, 32
C_OUT, KPAIRS = 512, 4
N_CORES = 8
CPC = C_OUT // N_CORES  # channels per core

P = 128          # partitions = (n=32) x (oyblk=4)
OYB = 4          # oy blocks per image
OYS = 8          # oy rows per block
HALO = 10        # rows stored per block (8 + 2 halo)
W34 = 34         # padded width
CHSZ = HALO * W34           # 340 elems per (q, channel)
XFREE = C_IN * CHSZ         # 21760 elems per partition
OUT_CSTRIDE = H * W * KPAIRS          # 4096
OUT_NSTRIDE = CPC * OUT_CSTRIDE       # 262144
SPP = OYS * W    # 256 elems per (partition, pair)

OP_COEFFS = np.array([
    [0.0, 0.0, 0.0, 0.0], [0.0, 0.0, 0.0, 1.0], [0.0, 1.0, 0.0, -1.0],
    [0.0, 1.0, 0.0, 0.0], [0.0, 0.0, 1.0, -1.0], [0.0, 0.0, 1.0, 0.0],
    [0.0, 1.0, 1.0, -2.0], [0.0, 1.0, 1.0, -1.0], [1.0, -1.0, -1.0, 1.0],
    [1.0, -1.0, -1.0, 2.0], [1.0, 0.0, -1.0, 0.0], [1.0, 0.0, -1.0, 1.0],
    [1.0, -1.0, 0.0, 0.0], [1.0, -1.0, 0.0, 1.0], [1.0, 0.0, 0.0, -1.0],
    [1.0, 0.0, 0.0, 0.0],
], dtype=np.float64)

MULT = mybir.AluOpType.mult
ADD = mybir.AluOpType.add
COPY = mybir.ActivationFunctionType.Copy
F16 = mybir.dt.float16

# TimelineSim per-op costs (fp16, [128,256] unless noted)
DVE_TS = 127.0    # tensor_scalar, 4x mode
DVE_TT = 194.0    # tensor_tensor contiguous, 2x mode
DVE_ATA = 327.0   # AFFINE_THEN_ADD custom uop (any stride)
ACT_TS = 398.0    # activation copy w/ scale+bias
GPS_TS = 451.0    # gpsimd tensor_scalar
GPS_TT_M = 2127.0   # gpsimd tensor_tensor [128,1024] merged y (stt is f32-only)
GPS_TT_S = 603.0    # gpsimd tensor_tensor [128,256] per-pair y
DVE_TT_M = 1127.0   # DVE tensor_tensor [128,1024] merged y (1x, strided in)

CFG = {
    "load_piece_ch": 4,  # channels per streamed load DMA
    "load_cascade": None,
    "tp_bufs": 8,    # u/w scratch tiles
    "wv_bufs": 5,    # w_all/v_all channel tiles (GPS merged-y inputs)
    "yc_bufs": 12,   # output channel tiles
    "pipe_depth": 0,  # channels of consumer delay (0 = inline)
    "head_ata": 0,   # first channels forced all-DVE
    "tail_ata": 0,   # last channels forced all-DVE
    "head_merged": 0,
    "gps_v_head": 0,  # first channels with v forced to GPSIMD
    "gps_single": 0,  # per-pair GPS y instead of per-channel merged
    "gps_half": 0,    # split merged y into two half-channel ops
    "auto_quota": 0,
    "f_ata": 0.28,    # fraction of channels on the all-DVE ATA path
    "uv_act": 0.445,  # fraction of merged-channel u/v ops on ACT
}

last_results = [None] * N_CORES  # BassKernelResults per core (for profiling)
last_model_ns = [None] * N_CORES  # per-core TimelineSim estimate
last_eng_ns = [None] * N_CORES  # per-core greedy engine-load projection


def core_channel_perm(core, ch):
    """Input-channel permutation for `core`: most-used first.

    With CFG['seed_channel'], the output channel needing the fewest distinct
    input channels gets its inputs at the very front, so its producer chain
    (and GPSIMD's first merged op) unlocks after the first 1-2 load pieces.

    Returns (perm, inv) with perm[pos] = original channel stored at slot pos
    in this core's xh, inv[orig] = slot.
    """
    use_cnt = [0] * C_IN
    need = []
    for cl in range(CPC):
        s = {int(ch[core * CPC + cl, j]) for j in range(2 * KPAIRS)}
        need.append(s)
        for j in range(2 * KPAIRS):
            use_cnt[int(ch[core * CPC + cl, j])] += 1
    head = []
    sc = CFG.get("seed_channel", 0)
    if isinstance(sc, (list, tuple)):
        sc = sc[core]
    if sc:
        best = min(range(CPC), key=lambda cl: len(need[cl]))
        head = sorted(need[best], key=lambda i: (-use_cnt[i], i))
    rest = [i for i in sorted(range(C_IN), key=lambda i: (-use_cnt[i], i)) if i not in set(head)]
    perm = head + rest
    inv = [0] * C_IN
    for pos, orig in enumerate(perm):
        inv[orig] = pos
    return perm, inv


def build_core_program(core, ch, ry, rx, coef):
    """One specialized Bass program for `core` (channels core*CPC..+CPC).

    The host stores this core's xh with channels permuted most-used-first
    (core_channel_perm); `ch` is remapped here to slot indices so the
    cascade [s0, s1, ...] loads contiguous slabs front-to-back.
    """
    nc = bacc.Bacc("TRN2", target_bir_lowering=False)
    xh_d = nc.dram_tensor("xh", [P, XFREE], F16, kind="ExternalInput")
    out_d = nc.dram_tensor(
        "out", [N, CPC, H, W, KPAIRS], F16, kind="ExternalOutput"
    )

    eng_ns = {
        "dve": float(CFG.get("seed_dve", 0.0)),
        "act": float(CFG.get("seed_act", 0.0)),
        "gps": float(CFG.get("seed_gps", 0.0)),
    }

    with TileContext(nc) as tc:
        with (
            tc.tile_pool(name="xp", bufs=1) as xpool,
            tc.tile_pool(name="tp", bufs=CFG["tp_bufs"]) as tpool,
            tc.tile_pool(name="wv", bufs=CFG["wv_bufs"]) as wvpool,
            tc.tile_pool(name="yp", bufs=CFG["yc_bufs"]) as ypool,
        ):
            xh = xpool.tile([P, XFREE], F16)
            # ch remapped to this core's permuted slots (most-used first).
            _, inv = core_channel_perm(core, ch)
            chv = {
                (cl, j): inv[int(ch[core * CPC + cl, j])]
                for cl in range(CPC)
                for j in range(2 * KPAIRS)
            }
            # Streamed load: many small contiguous piece-DMAs front-to-back
            # (most-used slots first).  Pairs/channels are ordered by the
            # piece at which both operands are resident, so the in-order
            # engine queues never head-of-line block on a late channel.
            sizes = CFG.get("load_cascade")
            if not sizes:
                g = CFG.get("load_piece_ch", 4)
                sizes = [g] * (C_IN // g) + ([C_IN % g] if C_IN % g else [])
            bounds = []
            acc = 0
            for sz in sizes:
                acc += sz
                bounds.append(acc)
            assert bounds[-1] == C_IN, bounds

            def tier_of_slot(slot):
                for t, b in enumerate(bounds):
                    if slot < b:
                        return t
                raise AssertionError(slot)

            ptier = {
                (cl, p4): max(
                    tier_of_slot(chv[(cl, 2 * p4)]), tier_of_slot(chv[(cl, 2 * p4 + 1)])
                )
                for cl in range(CPC)
                for p4 in range(KPAIRS)
            }
            tiers = {cl: sorted(ptier[(cl, p4)] for p4 in range(KPAIRS)) for cl in range(CPC)}
            # sort by completion tier (max first): a channel is consumable
            # only once its LAST pair's inputs arrive, so straggler channels
            # must not sit at the front of the in-order queues.
            cl_order = sorted(range(CPC), key=lambda cl: tiers[cl][::-1])
            p4_order = {
                cl: sorted(range(KPAIRS), key=lambda p4: ptier[(cl, p4)])
                for cl in range(CPC)
            }
            lo = 0
            for b in bounds:
                nc.sync.dma_start(
                    xh[:, lo * CHSZ : b * CHSZ], xh_d[:, lo * CHSZ : b * CHSZ]
                )
                lo = b
            base = xh[:]
            pitch = base.ap[0][0]
            tens = base.tensor
            base_off = base.offset

            # Software pipelining: emit each channel's producers (u/w/v) now
            # but its consumers (merged gstt + output DMA) D channels later,
            # so no engine's queue head waits on a just-issued cross-engine
            # dependency.
            pipe_d = CFG.get("pipe_depth", 3)
            pending = []  # (w_all, v_all, yc, oap, y_eng) awaiting merge+DMA
            st = {"mp": 0, "ua": 0, "va": 0}  # merged-pair / ACT-quota counters

            def flush_one():
                w_all, v_all, yc_t, oap_t, y_eng, pos_t = pending.pop(0)
                yb = yc_t[:]
                yp_ = yb.ap[0][0]
                wb = w_all[:]
                vb = v_all[:]
                halves = CFG.get("gps_half", 0) or pos_t < CFG.get("half_head", 0)
                nh = 2 if halves else 1
                pk = KPAIRS // nh
                for h in range(nh):
                    w_ap = bass.AP(wb.tensor, wb.offset + h * pk * SPP,
                                   [[wb.ap[0][0], P], [1, SPP], [SPP, pk]])
                    v_ap = bass.AP(vb.tensor, vb.offset + h * pk * SPP,
                                   [[vb.ap[0][0], P], [1, SPP], [SPP, pk]])
                    y_ap = bass.AP(yb.tensor, yb.offset + h * pk,
                                   [[yp_, P], [KPAIRS, SPP], [1, pk]])
                    if y_eng == "gps":
                        nc.gpsimd.tensor_tensor(y_ap, w_ap, v_ap, ADD)
                    else:
                        nc.vector.tensor_tensor(y_ap, w_ap, v_ap, ADD)
                nc.sync.dma_start(oap_t, yb)

            for cl in cl_order:
                c = core * CPC + cl
                yc = ypool.tile([P, OYS * W * KPAIRS], F16, tag="yc")
                ybase = yc[:]
                ypitch = ybase.ap[0][0]

                # channel mode: merged y on GPS vs per-pair ATA on DVE.
                # Project both and keep whichever minimizes the makespan.
                # LP-quota assignment: mode basis is m3 (u ACT + all-DVE ATA),
                # m5 (u/v DVE + y GPS-merged), m7 (u/v ACT + y GPS-merged).
                # Solved so each engine's start offset + load is equal --
                # engines finish together rather than having equal totals.
                pos = cl_order.index(cl)
                f_ata = CFG.get("f_ata", 0.281)
                uv_act = CFG.get("uv_act", 0.473)
                if CFG.get("auto_quota", 0):
                    M = np.array([
                        [521.0, 448.0, 194.0, -1.0],
                        [398.0, 0.0, 796.0, -1.0],
                        [0.0, 532.0, 532.0, -1.0],
                        [1.0, 1.0, 1.0, 0.0],
                    ])
                    rhs = np.array([
                        -float(CFG.get("off_dve", 6500.0)),
                        -float(CFG.get("off_act", 6500.0)),
                        -float(CFG.get("off_gps", 11000.0)),
                        float(C_OUT // N_CORES * KPAIRS),
                    ])
                    x3, x5, x7, _T = np.linalg.solve(M, rhs)
                    x3, x5, x7 = max(x3, 0.0), max(x5, 0.0), max(x7, 0.0)
                    tot = x3 + x5 + x7
                    f_ata = x3 / tot
                    uv_act = x7 / max(x5 + x7, 1e-9)
                n_ata = int(round(f_ata * CPC))
                tail = min(CFG.get("tail_ata", 0), n_ata)
                head_m = CFG.get("head_merged", 0)
                spread = n_ata - tail
                span = CPC - tail - head_m
                if pos >= CPC - tail:
                    is_ata = True
                elif pos < head_m or span <= 0:
                    is_ata = False
                else:
                    q = pos - head_m
                    is_ata = int(q * spread / span) < int((q + 1) * spread / span)
                if pos < CFG.get("head_ata", 0):
                    is_ata = True
                if is_ata:
                    mode = "ata"
                elif pos < CFG.get("dvem_head", 0):
                    # head channels' merged y on DVE: GPS is still ramping in,
                    # so this trims GPS's total without delaying its start.
                    mode = "dve_m"
                elif CFG.get("gps_single", 0):
                    mode = "gps_s"  # per-pair gtt, strided out (no w_all/v_all)
                else:
                    mode = "gps_m"
                merged = mode in ("gps_m", "dve_m")
                uplace, vplace = [], []
                for _ in range(KPAIRS):
                    if is_ata:
                        uplace.append("act")
                        eng_ns["act"] += ACT_TS
                        eng_ns["dve"] += DVE_TT + DVE_ATA
                        continue
                    if mode == "gps_s":
                        eng_ns["gps"] += GPS_TT_S
                    st["mp"] += 1
                    # u placement against the ACT quota
                    if st["ua"] < uv_act * st["mp"]:
                        uplace.append("act")
                        st["ua"] += 1
                        eng_ns["act"] += ACT_TS
                    else:
                        uplace.append("dve")
                        eng_ns["dve"] += DVE_TS
                    eng_ns["dve"] += DVE_TT
                    if pos < CFG.get("gps_v_head", 0):
                        vplace.append("gps")
                        eng_ns["gps"] += GPS_TS
                    elif st["va"] < uv_act * st["mp"]:
                        vplace.append("act")
                        st["va"] += 1
                        eng_ns["act"] += ACT_TS
                    else:
                        vplace.append("dve")
                        eng_ns["dve"] += DVE_TS
                if merged:
                    if mode == "dve_m":
                        eng_ns["dve"] += DVE_TT_M
                    else:
                        eng_ns["gps"] += 2222.0 if CFG.get("gps_half", 0) else GPS_TT_M
                    w_all = wvpool.tile([P, KPAIRS * SPP], F16, tag="wa")
                    v_all = wvpool.tile([P, KPAIRS * SPP], F16, tag="va")

                # hoist the first merged channels' producer chains so GPSIMD's
                # first merged op fires as early as possible
                hstack = contextlib.ExitStack()
                if merged and pos < CFG.get("hoist_head", 0):
                    hstack.enter_context(tc.high_priority())
                for i, p4 in enumerate(p4_order[cl]):
                    ka_, kb_ = 2 * p4, 2 * p4 + 1
                    offA = base_off + chv[(cl, ka_)] * CHSZ + int(ry[c, ka_]) * W34 + int(rx[c, ka_])
                    offB = base_off + chv[(cl, kb_)] * CHSZ + int(ry[c, kb_]) * W34 + int(rx[c, kb_])
                    A_ap = bass.AP(tens, offA, [[pitch, P], [W34, OYS], [1, W]])
                    B_ap = bass.AP(tens, offB, [[pitch, P], [W34, OYS], [1, W]])

                    k0 = float(coef[c, p4, 0])
                    ka = float(coef[c, p4, 1])
                    kb = float(coef[c, p4, 2])
                    kab = float(coef[c, p4, 3])

                    u = tpool.tile([P, SPP], F16, tag="u")
                    u3 = u[:].rearrange("p (a b) -> p a b", b=W)
                    # u = kab*B + ka
                    ue = uplace[i]
                    if ue == "act":
                        nc.scalar.activation(u3, B_ap, COPY, bias=ka, scale=kab)
                    elif ue == "gps":
                        nc.gpsimd.tensor_scalar(u3, B_ap, kab, ka, MULT, ADD)
                    else:
                        nc.vector.tensor_scalar(u3, B_ap, kab, ka, MULT, ADD)

                    if merged:
                        wsl = w_all[:, p4 * SPP : (p4 + 1) * SPP]
                        w3 = wsl.rearrange("p (a b) -> p a b", b=W)
                        # w = u*A  (DVE tt, 2x, contiguous out)
                        nc.vector.tensor_tensor(w3, u3, A_ap, MULT)
                        vsl = v_all[:, p4 * SPP : (p4 + 1) * SPP]
                        v3 = vsl.rearrange("p (a b) -> p a b", b=W)
                        e = vplace[i]
                        if e == "act":
                            nc.scalar.activation(v3, B_ap, COPY, bias=k0, scale=kb)
                        elif e == "gps":
                            nc.gpsimd.tensor_scalar(v3, B_ap, kb, k0, MULT, ADD)
                        else:
                            nc.vector.tensor_scalar(v3, B_ap, kb, k0, MULT, ADD)
                    elif mode == "gps_s":
                        w = tpool.tile([P, SPP], F16, tag="w")
                        v = tpool.tile([P, SPP], F16, tag="v")
                        w3 = w[:].rearrange("p (a b) -> p a b", b=W)
                        v3 = v[:].rearrange("p (a b) -> p a b", b=W)
                        nc.vector.tensor_tensor(w3, u3, A_ap, MULT)
                        e = vplace[i]
                        if e == "act":
                            nc.scalar.activation(v3, B_ap, COPY, bias=k0, scale=kb)
                        elif e == "gps":
                            nc.gpsimd.tensor_scalar(v3, B_ap, kb, k0, MULT, ADD)
                        else:
                            nc.vector.tensor_scalar(v3, B_ap, kb, k0, MULT, ADD)
                        yap = bass.AP(
                            ybase.tensor, ybase.offset + p4,
                            [[ypitch, P], [W * KPAIRS, OYS], [KPAIRS, W]],
                        )
                        nc.gpsimd.tensor_tensor(yap, w3, v3, ADD)
                    else:
                        w = tpool.tile([P, SPP], F16, tag="w")
                        w3 = w[:].rearrange("p (a b) -> p a b", b=W)
                        nc.vector.tensor_tensor(w3, u3, A_ap, MULT)
                        # y = (kb*B + k0) + w, p-interleaved into yc (custom uop)
                        yap = bass.AP(
                            ybase.tensor, ybase.offset + p4,
                            [[ypitch, P], [W * KPAIRS, OYS], [KPAIRS, W]],
                        )
                        nc.vector.affine_then_add(yap, B_ap, w3, kb, k0)

                hstack.close()
                # HBM [n, oyblk, (oy',ox,p)=1024]
                oap = bass.AP(
                    out_d, cl * OUT_CSTRIDE,
                    [[OUT_NSTRIDE, N], [OYS * W * KPAIRS, OYB], [1, OYS * W * KPAIRS]],
                )
                if merged:
                    pending.append((w_all, v_all, yc, oap, "gps" if mode == "gps_m" else "dve", pos))
                    if len(pending) > pipe_d:
                        flush_one()
                else:
                    nc.sync.dma_start(oap, ybase)
            while pending:
                flush_one()
    nc.finalize()  # Bacc: splits >1-wait syncs into event semaphores
    last_eng_ns[core] = dict(eng_ns)
    return nc


def _prep_inputs(x, weights, selection):
    x = np.ascontiguousarray(np.asarray(x, dtype=np.float32))
    weights = np.asarray(weights, dtype=np.float32)
    selection = np.asarray(selection, dtype=np.int32)

    # coefficients: softmax over 16 logic ops folded into {1,a,b,ab} basis
    w64 = weights.astype(np.float64)
    e = np.exp(w64 - w64.max(axis=-1, keepdims=True))
    prob = e / e.sum(axis=-1, keepdims=True)
    coef = (prob @ OP_COEFFS).astype(np.float32)  # [C_OUT, 4, 4]

    ch = ((selection >> 16) & 0xFFFF).astype(np.int64)
    ry = ((selection >> 8) & 0xFF).astype(np.int64)
    rx = (selection & 0xFF).astype(np.int64)

    # halo layout: xh[q=(n,oyblk), ch, r, w] = xpad[n, ch, oyblk*8+r, w]
    xpad = np.zeros((N, C_IN, H + 2, W + 2), dtype=np.float32)
    xpad[:, :, 1 : H + 1, 1 : W + 1] = x
    xh = np.empty((N, OYB, C_IN, HALO, W34), dtype=np.float16)
    for b in range(OYB):
        xh[:, b] = xpad[:, :, b * OYS : b * OYS + HALO, :]
    # per-core copies with channels permuted most-used-first so the load
    # cascade is a few big contiguous DMAs
    xh_cores = []
    for k in range(N_CORES):
        perm, _ = core_channel_perm(k, ch)
        xh_cores.append(np.ascontiguousarray(xh[:, :, perm].reshape(P, XFREE)))
    return xh_cores, ch, ry, rx, coef


def kernel(x, weights, selection):
    assert x.shape == (N, C_IN, H, W), x.shape
    assert weights.shape == (C_OUT, 4, 16), weights.shape
    assert selection.shape == (C_OUT, 8), selection.shape

    try:
        from concourse.timeline_sim import TimelineSim
    except Exception:  # noqa: BLE001
        TimelineSim = None

    # Pre-pass: decide the per-core seed-channel layout flag (it changes the
    # host xh layout, so it must be fixed before _prep_inputs and never
    # toggled by the per-core schedule candidates below).
    if TimelineSim is not None and not isinstance(CFG.get("seed_channel"), (list, tuple)):
        _, ch_t, ry_t, rx_t, coef_t = _prep_inputs(x, weights, selection)
        flags = []
        base_cfg0 = dict(CFG)
        for k in range(N_CORES):
            scores = {}
            for flag in (0, 1):
                best = None
                for delta in ({}, {"half_head": 2}):
                    CFG.clear()
                    CFG.update(base_cfg0)
                    CFG.update(delta)
                    CFG["seed_channel"] = flag
                    try:
                        ns = TimelineSim(
                            build_core_program(k, ch_t, ry_t, rx_t, coef_t),
                            trace=False,
                        ).simulate()
                    except Exception:  # noqa: BLE001
                        ns = float("inf")
                    best = ns if best is None else min(best, ns)
                scores[flag] = best
            flags.append(1 if scores[1] < scores[0] else 0)
        CFG.clear()
        CFG.update(base_cfg0)
        CFG["seed_channel"] = tuple(flags)

    xh_cores, ch, ry, rx, coef = _prep_inputs(x, weights, selection)
    # Per-core auto-tune: each core's selection pattern favors different
    # schedule knobs; build candidates and keep the TimelineSim-fastest.
    cands = CFG.get(
        "tune_candidates",
        (
            {},
            {"half_head": 2},
            {"load_piece_ch": 3, "yc_bufs": 13, "half_head": 1},
            {"f_ata": 0.283, "uv_act": 0.415, "gps_v_head": 1, "tail_ata": 1,
             "tp_bufs": 6, "wv_bufs": 8, "yc_bufs": 9},
            {"f_ata": 0.283, "uv_act": 0.395, "tail_ata": 1, "tp_bufs": 6,
             "wv_bufs": 8, "yc_bufs": 8},
            {"f_ata": 0.283, "uv_act": 0.395, "tail_ata": 1, "tp_bufs": 6,
             "wv_bufs": 8, "yc_bufs": 8, "half_head": 2},
            {"load_piece_ch": 6},
            {"f_ata": 0.26, "half_head": 2},
            {"f_ata": 0.31},
            {"uv_act": 0.40, "half_head": 2},
            {"uv_act": 0.50},
            {"uv_act": 0.395, "tail_ata": 1, "half_head": 2},
        ),
    )
    progs = []
    base_cfg = dict(CFG)
    for k in range(N_CORES):
        best = None
        for cfg_delta in cands if TimelineSim is not None else ({},):
            CFG.clear()
            CFG.update(base_cfg)
            CFG.update(cfg_delta)
            nc = build_core_program(k, ch, ry, rx, coef)
            ns = None
            if TimelineSim is not None:
                try:
                    ns = TimelineSim(nc, trace=False).simulate()
                except Exception:  # noqa: BLE001
                    ns = None
            if best is None or (ns is not None and best[0] is not None and ns < best[0]):
                best = (ns, nc)
            if ns is None:
                break
        progs.append(best[1])
        last_model_ns[k] = best[0]
    CFG.clear()
    CFG.update(base_cfg)

    import jax

    devices = jax.devices()
    assert len(devices) >= N_CORES, devices

    outs = [None] * N_CORES
    errs = [None] * N_CORES
    # NTFF tracing needs axon hooks that aren't present in this container —
    # make sure run_bass_kernel_spmd never tries (BASS_TRACE in env would).
    os.environ["BASS_NEVER_TRACE"] = "1"

    def run_one(k):
        try:
            with jax.default_device(devices[k]):
                res = bass_utils.run_bass_kernel_spmd(
                    progs[k], [{"xh": xh_cores[k]}], core_ids=[k]
                )
            last_results[k] = res
            outs[k] = res.results[0]["out"]
        except Exception as e:  # noqa: BLE001
            errs[k] = e

    threads = [threading.Thread(target=run_one, args=(k,)) for k in range(N_CORES)]
    for t in threads:
        t.start()
    for t in threads:
        t.join()
    for k, e in enumerate(errs):
        if e is not None:
            raise RuntimeError(f"core {k} failed") from e

    y = np.empty((N, C_OUT, H, W, KPAIRS), dtype=np.float32)
    for k in range(N_CORES):
        y[:, k * CPC : (k + 1) * CPC] = np.asarray(outs[k], dtype=np.float32)
    return y


# revision 63
# speedup vs baseline: 1.4567x; 1.0001x over previous
"""Trainium2 Bass kernel for nn_ConvLogicLayer.

Computes y[n,c,oy,ox,p] = k0 + ka*A + kb*B + kab*A*B where A/B are
shifted-window gathers of input channels (per the packed `selection`),
and k* are per-(c,p) coefficients derived from softmax(weights) @ OP_COEFFS.

Strategy (v2, fp16, multi-engine LP balance):
  - Shard C_out (512) across 8 cores -> 64 output channels per core.
  - Specialized per-core program: gather indices and coefficients baked
    into the instruction stream (static APs + immediate scalars).
  - SBUF layout: partition q = n*4 + oyblk (32 images x 4 row-blocks),
    free dim = 64 input channels x 10 halo rows x 34 padded cols, fp16,
    channels permuted per-core most-used-first.  A shifted 8x32 window
    for any (ch,ry,rx) is a static 3D AP.  The load streams as small
    contiguous piece-DMAs; channels are processed in completion-tier
    order so the in-order engine queues never wait on a late channel.
  - fp16 everywhere: DVE tensor_scalar runs in 4x mode (127ns/[128,256]),
    tensor_tensor in 2x mode (194ns), DMA bytes halve (in 5.6MB, out
    16.8MB per core).  Host converts the fp16 output to f32; rel err vs
    the f32 reference is ~1e-3 (tolerance 2e-2).
  - Per pair (c,p): u = kab*B + ka (DVE ts or ACT activation), then
    w = u*A (DVE tt).  y two ways, split by an offline LP over the
    per-op engine costs (DVE/ACT/GPSIMD finish together, ~98us loads):
      * f_ata of channels: per-pair y = (kb*B + k0) + w via the
        AFFINE_THEN_ADD custom DVE uop (strided p-interleaved write,
        v folded free; u on ACT for these channels);
      * the rest: v = kb*B + k0 (uv_act of u/v on ACT, rest DVE) into
        v_all, then ONE GPSIMD tensor_tensor [128,1024] per channel
        computes yc = w_all + v_all with the p-interleave expressed in
        the APs.  (scalar_tensor_tensor on Pool is f32-only in the real
        backend -- tensor_tensor ADD is the fp16-legal form.)
  - y written p-interleaved so the per-channel output DMA (256KB fp16)
    is 2KB-contiguous per partition in HBM and overlaps compute.
  - Per-core auto-tune over schedule knobs via TimelineSim.
"""

import contextlib
import os
import sys
import threading

import numpy as np

for _p in ("/opt/trn_rl_repo",):
    if _p not in sys.path and os.path.isdir(_p):
        sys.path.insert(0, _p)

import concourse.bass as bass
import concourse.bacc as bacc
import concourse.mybir as mybir
from concourse.tile import TileContext
from concourse import bass_utils

# Problem constants (hardcoded per spec)
N, C_IN, H, W = 32, 64, 32, 32
C_OUT, KPAIRS = 512, 4
N_CORES = 8
CPC = C_OUT // N_CORES  # channels per core

P = 128          # partitions = (n=32) x (oyblk=4)
OYB = 4          # oy blocks per image
OYS = 8          # oy rows per block
HALO = 10        # rows stored per block (8 + 2 halo)
W34 = 34         # padded width
CHSZ = HALO * W34           # 340 elems per (q, channel)
XFREE = C_IN * CHSZ         # 21760 elems per partition
OUT_CSTRIDE = H * W * KPAIRS          # 4096
OUT_NSTRIDE = CPC * OUT_CSTRIDE       # 262144
SPP = OYS * W    # 256 elems per (partition, pair)

OP_COEFFS = np.array([
    [0.0, 0.0, 0.0, 0.0], [0.0, 0.0, 0.0, 1.0], [0.0, 1.0, 0.0, -1.0],
    [0.0, 1.0, 0.0, 0.0], [0.0, 0.0, 1.0, -1.0], [0.0, 0.0, 1.0, 0.0],
    [0.0, 1.0, 1.0, -2.0], [0.0, 1.0, 1.0, -1.0], [1.0, -1.0, -1.0, 1.0],
    [1.0, -1.0, -1.0, 2.0], [1.0, 0.0, -1.0, 0.0], [1.0, 0.0, -1.0, 1.0],
    [1.0, -1.0, 0.0, 0.0], [1.0, -1.0, 0.0, 1.0], [1.0, 0.0, 0.0, -1.0],
    [1.0, 0.0, 0.0, 0.0],
], dtype=np.float64)

MULT = mybir.AluOpType.mult
ADD = mybir.AluOpType.add
COPY = mybir.ActivationFunctionType.Copy
F16 = mybir.dt.float16

# TimelineSim per-op costs (fp16, [128,256] unless noted)
DVE_TS = 127.0    # tensor_scalar, 4x mode
DVE_TT = 194.0    # tensor_tensor contiguous, 2x mode
DVE_ATA = 327.0   # AFFINE_THEN_ADD custom uop (any stride)
ACT_TS = 398.0    # activation copy w/ scale+bias
GPS_TS = 451.0    # gpsimd tensor_scalar
GPS_TT_M = 2127.0   # gpsimd tensor_tensor [128,1024] merged y (stt is f32-only)
GPS_TT_S = 603.0    # gpsimd tensor_tensor [128,256] per-pair y
DVE_TT_M = 1127.0   # DVE tensor_tensor [128,1024] merged y (1x, strided in)

CFG = {
    "load_piece_ch": 4,  # channels per streamed load DMA
    "load_cascade": None,
    "tp_bufs": 8,    # u/w scratch tiles
    "wv_bufs": 5,    # w_all/v_all channel tiles (GPS merged-y inputs)
    "yc_bufs": 12,   # output channel tiles
    "pipe_depth": 0,  # channels of consumer delay (0 = inline)
    "head_ata": 0,   # first channels forced all-DVE
    "tail_ata": 0,   # last channels forced all-DVE
    "head_merged": 0,
    "gps_v_head": 0,  # first channels with v forced to GPSIMD
    "gps_single": 0,  # per-pair GPS y instead of per-channel merged
    "gps_half": 0,    # split merged y into two half-channel ops
    "auto_quota": 0,
    "f_ata": 0.28,    # fraction of channels on the all-DVE ATA path
    "uv_act": 0.445,  # fraction of merged-channel u/v ops on ACT
}

last_results = [None] * N_CORES  # BassKernelResults per core (for profiling)
last_model_ns = [None] * N_CORES  # per-core TimelineSim estimate
last_eng_ns = [None] * N_CORES  # per-core greedy engine-load projection


def core_channel_perm(core, ch):
    """Input-channel permutation for `core`: most-used first.

    With CFG['seed_channel'], the output channel needing the fewest distinct
    input channels gets its inputs at the very front, so its producer chain
    (and GPSIMD's first merged op) unlocks after the first 1-2 load pieces.

    Returns (perm, inv) with perm[pos] = original channel stored at slot pos
    in this core's xh, inv[orig] = slot.
    """
    use_cnt = [0] * C_IN
    need = []
    for cl in range(CPC):
        s = {int(ch[core * CPC + cl, j]) for j in range(2 * KPAIRS)}
        need.append(s)
        for j in range(2 * KPAIRS):
            use_cnt[int(ch[core * CPC + cl, j])] += 1
    head = []
    sc = CFG.get("seed_channel", 0)
    if isinstance(sc, (list, tuple)):
        sc = sc[core]
    if sc:
        best = min(range(CPC), key=lambda cl: len(need[cl]))
        head = sorted(need[best], key=lambda i: (-use_cnt[i], i))
    rest = [i for i in sorted(range(C_IN), key=lambda i: (-use_cnt[i], i)) if i not in set(head)]
    perm = head + rest
    inv = [0] * C_IN
    for pos, orig in enumerate(perm):
        inv[orig] = pos
    return perm, inv


def build_core_program(core, ch, ry, rx, coef):
    """One specialized Bass program for `core` (channels core*CPC..+CPC).

    The host stores this core's xh with channels permuted most-used-first
    (core_channel_perm); `ch` is remapped here to slot indices so the
    cascade [s0, s1, ...] loads contiguous slabs front-to-back.
    """
    nc = bacc.Bacc("TRN2", target_bir_lowering=False)
    xh_d = nc.dram_tensor("xh", [P, XFREE], F16, kind="ExternalInput")
    out_d = nc.dram_tensor(
        "out", [N, CPC, H, W, KPAIRS], F16, kind="ExternalOutput"
    )

    eng_ns = {
        "dve": float(CFG.get("seed_dve", 0.0)),
        "act": float(CFG.get("seed_act", 0.0)),
        "gps": float(CFG.get("seed_gps", 0.0)),
    }

    with TileContext(nc) as tc:
        with (
            tc.tile_pool(name="xp", bufs=1) as xpool,
            tc.tile_pool(name="tp", bufs=CFG["tp_bufs"]) as tpool,
            tc.tile_pool(name="wv", bufs=CFG["wv_bufs"]) as wvpool,
            tc.tile_pool(name="yp", bufs=CFG["yc_bufs"]) as ypool,
        ):
            xh = xpool.tile([P, XFREE], F16)
            # ch remapped to this core's permuted slots (most-used first).
            _, inv = core_channel_perm(core, ch)
            chv = {
                (cl, j): inv[int(ch[core * CPC + cl, j])]
                for cl in range(CPC)
                for j in range(2 * KPAIRS)
            }
            # Streamed load: many small contiguous piece-DMAs front-to-back
            # (most-used slots first).  Pairs/channels are ordered by the
            # piece at which both operands are resident, so the in-order
            # engine queues never head-of-line block on a late channel.
            sizes = CFG.get("load_cascade")
            if not sizes:
                g = CFG.get("load_piece_ch", 4)
                sizes = [g] * (C_IN // g) + ([C_IN % g] if C_IN % g else [])
            bounds = []
            acc = 0
            for sz in sizes:
                acc += sz
                bounds.append(acc)
            assert bounds[-1] == C_IN, bounds

            def tier_of_slot(slot):
                for t, b in enumerate(bounds):
                    if slot < b:
                        return t
                raise AssertionError(slot)

            ptier = {
                (cl, p4): max(
                    tier_of_slot(chv[(cl, 2 * p4)]), tier_of_slot(chv[(cl, 2 * p4 + 1)])
                )
                for cl in range(CPC)
                for p4 in range(KPAIRS)
            }
            tiers = {cl: sorted(ptier[(cl, p4)] for p4 in range(KPAIRS)) for cl in range(CPC)}
            # sort by completion tier (max first): a channel is consumable
            # only once its LAST pair's inputs arrive, so straggler channels
            # must not sit at the front of the in-order queues.
            cl_order = sorted(range(CPC), key=lambda cl: tiers[cl][::-1])
            p4_order = {
                cl: sorted(range(KPAIRS), key=lambda p4: ptier[(cl, p4)])
                for cl in range(CPC)
            }
            lo = 0
            for b in bounds:
                nc.sync.dma_start(
                    xh[:, lo * CHSZ : b * CHSZ], xh_d[:, lo * CHSZ : b * CHSZ]
                )
                lo = b
            base = xh[:]
            pitch = base.ap[0][0]
            tens = base.tensor
            base_off = base.offset

            # Software pipelining: emit each channel's producers (u/w/v) now
            # but its consumers (merged gstt + output DMA) D channels later,
            # so no engine's queue head waits on a just-issued cross-engine
            # dependency.
            pipe_d = CFG.get("pipe_depth", 3)
            pending = []  # (w_all, v_all, yc, oap, y_eng) awaiting merge+DMA
            st = {"mp": 0, "ua": 0, "va": 0}  # merged-pair / ACT-quota counters

            def flush_one():
                w_all, v_all, yc_t, oap_t, y_eng, pos_t = pending.pop(0)
                yb = yc_t[:]
                yp_ = yb.ap[0][0]
                wb = w_all[:]
                vb = v_all[:]
                halves = CFG.get("gps_half", 0) or pos_t < CFG.get("half_head", 0)
                nh = 2 if halves else 1
                pk = KPAIRS // nh
                for h in range(nh):
                    w_ap = bass.AP(wb.tensor, wb.offset + h * pk * SPP,
                                   [[wb.ap[0][0], P], [1, SPP], [SPP, pk]])
                    v_ap = bass.AP(vb.tensor, vb.offset + h * pk * SPP,
                                   [[vb.ap[0][0], P], [1, SPP], [SPP, pk]])
                    y_ap = bass.AP(yb.tensor, yb.offset + h * pk,
                                   [[yp_, P], [KPAIRS, SPP], [1, pk]])
                    if y_eng == "gps":
                        nc.gpsimd.tensor_tensor(y_ap, w_ap, v_ap, ADD)
                    else:
                        nc.vector.tensor_tensor(y_ap, w_ap, v_ap, ADD)
                nc.sync.dma_start(oap_t, yb)

            for cl in cl_order:
                c = core * CPC + cl
                yc = ypool.tile([P, OYS * W * KPAIRS], F16, tag="yc")
                ybase = yc[:]
                ypitch = ybase.ap[0][0]

                # channel mode: merged y on GPS vs per-pair ATA on DVE.
                # Project both and keep whichever minimizes the makespan.
                # LP-quota assignment: mode basis is m3 (u ACT + all-DVE ATA),
                # m5 (u/v DVE + y GPS-merged), m7 (u/v ACT + y GPS-merged).
                # Solved so each engine's start offset + load is equal --
                # engines finish together rather than having equal totals.
                pos = cl_order.index(cl)
                f_ata = CFG.get("f_ata", 0.281)
                uv_act = CFG.get("uv_act", 0.473)
                if CFG.get("auto_quota", 0):
                    M = np.array([
                        [521.0, 448.0, 194.0, -1.0],
                        [398.0, 0.0, 796.0, -1.0],
                        [0.0, 532.0, 532.0, -1.0],
                        [1.0, 1.0, 1.0, 0.0],
                    ])
                    rhs = np.array([
                        -float(CFG.get("off_dve", 6500.0)),
                        -float(CFG.get("off_act", 6500.0)),
                        -float(CFG.get("off_gps", 11000.0)),
                        float(C_OUT // N_CORES * KPAIRS),
                    ])
                    x3, x5, x7, _T = np.linalg.solve(M, rhs)
                    x3, x5, x7 = max(x3, 0.0), max(x5, 0.0), max(x7, 0.0)
                    tot = x3 + x5 + x7
                    f_ata = x3 / tot
                    uv_act = x7 / max(x5 + x7, 1e-9)
                n_ata = int(round(f_ata * CPC))
                tail = min(CFG.get("tail_ata", 0), n_ata)
                head_m = CFG.get("head_merged", 0)
                spread = n_ata - tail
                span = CPC - tail - head_m
                if pos >= CPC - tail:
                    is_ata = True
                elif pos < head_m or span <= 0:
                    is_ata = False
                else:
                    q = pos - head_m
                    is_ata = int(q * spread / span) < int((q + 1) * spread / span)
                if pos < CFG.get("head_ata", 0):
                    is_ata = True
                if is_ata:
                    mode = "ata"
                elif pos < CFG.get("dvem_head", 0):
                    # head channels' merged y on DVE: GPS is still ramping in,
                    # so this trims GPS's total without delaying its start.
                    mode = "dve_m"
                elif CFG.get("gps_single", 0):
                    mode = "gps_s"  # per-pair gtt, strided out (no w_all/v_all)
                else:
                    mode = "gps_m"
                merged = mode in ("gps_m", "dve_m")
                uplace, vplace = [], []
                for _ in range(KPAIRS):
                    if is_ata:
                        uplace.append("act")
                        eng_ns["act"] += ACT_TS
                        eng_ns["dve"] += DVE_TT + DVE_ATA
                        continue
                    if mode == "gps_s":
                        eng_ns["gps"] += GPS_TT_S
                    st["mp"] += 1
                    # u placement against the ACT quota
                    if st["ua"] < uv_act * st["mp"]:
                        uplace.append("act")
                        st["ua"] += 1
                        eng_ns["act"] += ACT_TS
                    else:
                        uplace.append("dve")
                        eng_ns["dve"] += DVE_TS
                    eng_ns["dve"] += DVE_TT
                    if pos < CFG.get("gps_v_head", 0):
                        vplace.append("gps")
                        eng_ns["gps"] += GPS_TS
                    elif st["va"] < uv_act * st["mp"]:
                        vplace.append("act")
                        st["va"] += 1
                        eng_ns["act"] += ACT_TS
                    else:
                        vplace.append("dve")
                        eng_ns["dve"] += DVE_TS
                if merged:
                    if mode == "dve_m":
                        eng_ns["dve"] += DVE_TT_M
                    else:
                        eng_ns["gps"] += 2222.0 if CFG.get("gps_half", 0) else GPS_TT_M
                    w_all = wvpool.tile([P, KPAIRS * SPP], F16, tag="wa")
                    v_all = wvpool.tile([P, KPAIRS * SPP], F16, tag="va")

                # hoist the first merged channels' producer chains so GPSIMD's
                # first merged op fires as early as possible
                hstack = contextlib.ExitStack()
                if merged and pos < CFG.get("hoist_head", 0):
                    hstack.enter_context(tc.high_priority())
                for i, p4 in enumerate(p4_order[cl]):
                    ka_, kb_ = 2 * p4, 2 * p4 + 1
                    offA = base_off + chv[(cl, ka_)] * CHSZ + int(ry[c, ka_]) * W34 + int(rx[c, ka_])
                    offB = base_off + chv[(cl, kb_)] * CHSZ + int(ry[c, kb_]) * W34 + int(rx[c, kb_])
                    A_ap = bass.AP(tens, offA, [[pitch, P], [W34, OYS], [1, W]])
                    B_ap = bass.AP(tens, offB, [[pitch, P], [W34, OYS], [1, W]])

                    k0 = float(coef[c, p4, 0])
                    ka = float(coef[c, p4, 1])
                    kb = float(coef[c, p4, 2])
                    kab = float(coef[c, p4, 3])

                    u = tpool.tile([P, SPP], F16, tag="u")
                    u3 = u[:].rearrange("p (a b) -> p a b", b=W)
                    # u = kab*B + ka
                    ue = uplace[i]
                    if ue == "act":
                        nc.scalar.activation(u3, B_ap, COPY, bias=ka, scale=kab)
                    elif ue == "gps":
                        nc.gpsimd.tensor_scalar(u3, B_ap, kab, ka, MULT, ADD)
                    else:
                        nc.vector.tensor_scalar(u3, B_ap, kab, ka, MULT, ADD)

                    if merged:
                        wsl = w_all[:, p4 * SPP : (p4 + 1) * SPP]
                        w3 = wsl.rearrange("p (a b) -> p a b", b=W)
                        # w = u*A  (DVE tt, 2x, contiguous out)
                        nc.vector.tensor_tensor(w3, u3, A_ap, MULT)
                        vsl = v_all[:, p4 * SPP : (p4 + 1) * SPP]
                        v3 = vsl.rearrange("p (a b) -> p a b", b=W)
                        e = vplace[i]
                        if e == "act":
                            nc.scalar.activation(v3, B_ap, COPY, bias=k0, scale=kb)
                        elif e == "gps":
                            nc.gpsimd.tensor_scalar(v3, B_ap, kb, k0, MULT, ADD)
                        else:
                            nc.vector.tensor_scalar(v3, B_ap, kb, k0, MULT, ADD)
                    elif mode == "gps_s":
                        w = tpool.tile([P, SPP], F16, tag="w")
                        v = tpool.tile([P, SPP], F16, tag="v")
                        w3 = w[:].rearrange("p (a b) -> p a b", b=W)
                        v3 = v[:].rearrange("p (a b) -> p a b", b=W)
                        nc.vector.tensor_tensor(w3, u3, A_ap, MULT)
                        e = vplace[i]
                        if e == "act":
                            nc.scalar.activation(v3, B_ap, COPY, bias=k0, scale=kb)
                        elif e == "gps":
                            nc.gpsimd.tensor_scalar(v3, B_ap, kb, k0, MULT, ADD)
                        else:
                            nc.vector.tensor_scalar(v3, B_ap, kb, k0, MULT, ADD)
                        yap = bass.AP(
                            ybase.tensor, ybase.offset + p4,
                            [[ypitch, P], [W * KPAIRS, OYS], [KPAIRS, W]],
                        )
                        nc.gpsimd.tensor_tensor(yap, w3, v3, ADD)
                    else:
                        w = tpool.tile([P, SPP], F16, tag="w")
                        w3 = w[:].rearrange("p (a b) -> p a b", b=W)
                        nc.vector.tensor_tensor(w3, u3, A_ap, MULT)
                        # y = (kb*B + k0) + w, p-interleaved into yc (custom uop)
                        yap = bass.AP(
                            ybase.tensor, ybase.offset + p4,
                            [[ypitch, P], [W * KPAIRS, OYS], [KPAIRS, W]],
                        )
                        nc.vector.affine_then_add(yap, B_ap, w3, kb, k0)

                hstack.close()
                # HBM [n, oyblk, (oy',ox,p)=1024]
                oap = bass.AP(
                    out_d, cl * OUT_CSTRIDE,
                    [[OUT_NSTRIDE, N], [OYS * W * KPAIRS, OYB], [1, OYS * W * KPAIRS]],
                )
                if merged:
                    pending.append((w_all, v_all, yc, oap, "gps" if mode == "gps_m" else "dve", pos))
                    if len(pending) > pipe_d:
                        flush_one()
                else:
                    nc.sync.dma_start(oap, ybase)
            while pending:
                flush_one()
    nc.finalize()  # Bacc: splits >1-wait syncs into event semaphores
    last_eng_ns[core] = dict(eng_ns)
    return nc


def _prep_inputs(x, weights, selection):
    x = np.ascontiguousarray(np.asarray(x, dtype=np.float32))
    weights = np.asarray(weights, dtype=np.float32)
    selection = np.asarray(selection, dtype=np.int32)

    # coefficients: softmax over 16 logic ops folded into {1,a,b,ab} basis
    w64 = weights.astype(np.float64)
    e = np.exp(w64 - w64.max(axis=-1, keepdims=True))
    prob = e / e.sum(axis=-1, keepdims=True)
    coef = (prob @ OP_COEFFS).astype(np.float32)  # [C_OUT, 4, 4]

    ch = ((selection >> 16) & 0xFFFF).astype(np.int64)
    ry = ((selection >> 8) & 0xFF).astype(np.int64)
    rx = (selection & 0xFF).astype(np.int64)

    # halo layout: xh[q=(n,oyblk), ch, r, w] = xpad[n, ch, oyblk*8+r, w]
    xpad = np.zeros((N, C_IN, H + 2, W + 2), dtype=np.float32)
    xpad[:, :, 1 : H + 1, 1 : W + 1] = x
    xh = np.empty((N, OYB, C_IN, HALO, W34), dtype=np.float16)
    for b in range(OYB):
        xh[:, b] = xpad[:, :, b * OYS : b * OYS + HALO, :]
    # per-core copies with channels permuted most-used-first so the load
    # cascade is a few big contiguous DMAs
    xh_cores = []
    for k in range(N_CORES):
        perm, _ = core_channel_perm(k, ch)
        xh_cores.append(np.ascontiguousarray(xh[:, :, perm].reshape(P, XFREE)))
    return xh_cores, ch, ry, rx, coef


def kernel(x, weights, selection):
    assert x.shape == (N, C_IN, H, W), x.shape
    assert weights.shape == (C_OUT, 4, 16), weights.shape
    assert selection.shape == (C_OUT, 8), selection.shape

    try:
        from concourse.timeline_sim import TimelineSim
    except Exception:  # noqa: BLE001
        TimelineSim = None

    # Pre-pass: decide the per-core seed-channel layout flag (it changes the
    # host xh layout, so it must be fixed before _prep_inputs and never
    # toggled by the per-core schedule candidates below).
    if TimelineSim is not None and not isinstance(CFG.get("seed_channel"), (list, tuple)):
        _, ch_t, ry_t, rx_t, coef_t = _prep_inputs(x, weights, selection)
        flags = []
        base_cfg0 = dict(CFG)
        for k in range(N_CORES):
            scores = {}
            for flag in (0, 1):
                best = None
                for delta in ({}, {"half_head": 2}):
                    CFG.clear()
                    CFG.update(base_cfg0)
                    CFG.update(delta)
                    CFG["seed_channel"] = flag
                    try:
                        ns = TimelineSim(
                            build_core_program(k, ch_t, ry_t, rx_t, coef_t),
                            trace=False,
                        ).simulate()
                    except Exception:  # noqa: BLE001
                        ns = float("inf")
                    best = ns if best is None else min(best, ns)
                scores[flag] = best
            flags.append(1 if scores[1] < scores[0] else 0)
        CFG.clear()
        CFG.update(base_cfg0)
        CFG["seed_channel"] = tuple(flags)

    xh_cores, ch, ry, rx, coef = _prep_inputs(x, weights, selection)
    # Per-core auto-tune: each core's selection pattern favors different
    # schedule knobs; build candidates and keep the TimelineSim-fastest.
    cands = CFG.get(
        "tune_candidates",
        (
            {},
            {"half_head": 2},
            {"load_piece_ch": 3, "yc_bufs": 13, "half_head": 1},
            {"f_ata": 0.283, "uv_act": 0.415, "gps_v_head": 1, "tail_ata": 1,
             "tp_bufs": 6, "wv_bufs": 8, "yc_bufs": 9},
            {"f_ata": 0.283, "uv_act": 0.395, "tail_ata": 1, "tp_bufs": 6,
             "wv_bufs": 8, "yc_bufs": 8},
            {"f_ata": 0.283, "uv_act": 0.395, "tail_ata": 1, "tp_bufs": 6,
             "wv_bufs": 8, "yc_bufs": 8, "half_head": 2},
            {"f_ata": 0.283, "uv_act": 0.395, "tail_ata": 1, "tp_bufs": 6,
             "wv_bufs": 8, "yc_bufs": 8, "half_head": 8},
            {"load_piece_ch": 6},
            {"f_ata": 0.26, "half_head": 2},
            {"f_ata": 0.31},
            {"uv_act": 0.40, "half_head": 2},
            {"uv_act": 0.50},
            {"uv_act": 0.395, "tail_ata": 1, "half_head": 2},
        ),
    )
    progs = []
    base_cfg = dict(CFG)
    for k in range(N_CORES):
        best = None
        for cfg_delta in cands if TimelineSim is not None else ({},):
            CFG.clear()
            CFG.update(base_cfg)
            CFG.update(cfg_delta)
            nc = build_core_program(k, ch, ry, rx, coef)
            ns = None
            if TimelineSim is not None:
                try:
                    ns = TimelineSim(nc, trace=False).simulate()
                except Exception:  # noqa: BLE001
                    ns = None
            if best is None or (ns is not None and best[0] is not None and ns < best[0]):
                best = (ns, nc)
            if ns is None:
                break
        progs.append(best[1])
        last_model_ns[k] = best[0]
    CFG.clear()
    CFG.update(base_cfg)

    import jax

    devices = jax.devices()
    assert len(devices) >= N_CORES, devices

    outs = [None] * N_CORES
    errs = [None] * N_CORES
    # NTFF tracing needs axon hooks that aren't present in this container —
    # make sure run_bass_kernel_spmd never tries (BASS_TRACE in env would).
    os.environ["BASS_NEVER_TRACE"] = "1"

    def run_one(k):
        try:
            with jax.default_device(devices[k]):
                res = bass_utils.run_bass_kernel_spmd(
                    progs[k], [{"xh": xh_cores[k]}], core_ids=[k]
                )
            last_results[k] = res
            outs[k] = res.results[0]["out"]
        except Exception as e:  # noqa: BLE001
            errs[k] = e

    threads = [threading.Thread(target=run_one, args=(k,)) for k in range(N_CORES)]
    for t in threads:
        t.start()
    for t in threads:
        t.join()
    for k, e in enumerate(errs):
        if e is not None:
            raise RuntimeError(f"core {k} failed") from e

    y = np.empty((N, C_OUT, H, W, KPAIRS), dtype=np.float32)
    for k in range(N_CORES):
        y[:, k * CPC : (k + 1) * CPC] = np.asarray(outs[k], dtype=np.float32)
    return y
